# revision 1
# baseline (speedup 1.0000x reference)
"""Trainium2 Bass kernel for nn_Commnet (gnn_message_passing).

kernel(**inputs) takes FULL unsharded numpy inputs, returns (logp [4096,16],)
computed across 8 NeuronCores (SPMD single program; per-core structure is
carried entirely in input tensors).

Sharding: batches bin-packed into 32 sub-shards (4/core); each sub-shard =
10 batch-groups of 16 batch slots padded to exactly 512 agent slots, so every
512-agent matmul block has a static 16-batch selector window -> the program
is identical on all cores.

EmbeddingBag(mean): table cast to bf16 pre-scaled by 1/8 on host (exact);
bulk SWDGE dma_gather (int16 idx wrapped in 16 partitions, replicated
across the 8 Q7 cores) fetches the 8 word rows per agent; the bag-sum and
the agent->dim transpose are fused as 8 accumulating PE matmuls against a
bf16 identity (fp32 PSUM accumulate), one per word slot.

Middle layers: emb' = relu(W_l@emb - W_r@(emb*recip) + R'@sel) where
R' = W_r@m + b (x) (len-0.99999) folds expansion+bias; sel is a banded
one-hot*recip selector (1 extra k-tile per block). m = segment sum via DVE
group-sum(8) -> PE transpose -> banded matmuls into disjoint PSUM windows.
Padded agents stay exactly 0 through all layers.
"""

from contextlib import ExitStack

import numpy as np
import ml_dtypes

import concourse.bass as bass
import concourse.bacc as bacc
import concourse.tile as tile
from concourse import mybir
from concourse.masks import make_identity

N_WORDS = 32000
EDIM = 256
N_AGENTS = 131072
BAG = 8
N_BATCHES = 4096
N_ACTIONS = 16
NLAYERS = 3

N_CORES = 8
NSUB = 4                  # sub-shards per core
NSHARD = N_CORES * NSUB   # 32
NBG = 10                  # batch-groups per sub-shard
BG_BATCHES = 16
BG_AGENTS = 512
B_S = NBG * BG_BATCHES    # 160
A_S = NBG * BG_AGENTS     # 5120
G = 8
NG = A_S // G             # 640
NCHUNK = NG // 128        # 5
W2 = 2 * BG_BATCHES       # 32
P = 128
ZPAD_ROW = N_WORDS
DT = mybir.dt
AF = mybir.ActivationFunctionType
OP = mybir.AluOpType

_PROGRAM_CACHE = {}
_DEBUG = False


# ================================================================ host prep
def _pack_batches(counts):
    padded = ((counts + G - 1) // G) * G
    ngroups = NSHARD * NBG
    cap = np.full(ngroups, BG_AGENTS, dtype=np.int64)
    slots = np.full(ngroups, BG_BATCHES, dtype=np.int64)
    members = [[] for _ in range(ngroups)]
    for b in np.argsort(-padded, kind="stable"):
        ok = (cap >= padded[b]) & (slots > 0)
        if not ok.any():
            raise RuntimeError("bin packing failed")
        g = int(np.argmax(np.where(ok, cap, -1)))
        members[g].append(int(b))
        cap[g] -= padded[b]
        slots[g] -= 1
    return members, padded


def _build_host_inputs(x, batch_idx, batch_len, emb_table, W0, b0, W1, b1,
                       Wh, bh):
    bf16 = ml_dtypes.bfloat16
    x = np.asarray(x, dtype=np.int64)
    batch_idx = np.asarray(batch_idx, dtype=np.int64)
    batch_len64 = np.asarray(batch_len, dtype=np.float64)

    counts = np.bincount(batch_idx, minlength=N_BATCHES).astype(np.int64)
    starts = np.concatenate([[0], np.cumsum(counts)[:-1]])
    members, padded = _pack_batches(counts)

    # dma_gather int16 indices: per (word k, agent-chunk q of 512), a
    # [16, 32] block at column offset (k*NBG+q)*32; stream position i
    # (agent q*512+i) lives at [i%16, i//16].
    idx_all = np.zeros((NSHARD, P, BAG * NBG * 32), dtype=np.int16)
    xsel_all = np.zeros((NSHARD, P, A_S), dtype=bf16)
    recip_all = np.zeros((NSHARD, P, A_S), dtype=bf16)
    s2_all = np.zeros((NSHARD, P, NCHUNK, W2), dtype=bf16)
    lenm1_all = np.zeros((NSHARD, 1, B_S), dtype=bf16)
    gcnt_all = np.full((NSHARD, 1, NBG // 2), BG_AGENTS * 2, dtype=np.int32)
    out_map = np.full((NSHARD, B_S), -1, dtype=np.int64)
    dbg_slots = []
    recip_f = (1.0 / (batch_len64 - 0.99999)).astype(np.float32)

    for sh in range(NSHARD):
        idx_flat = np.full((BAG, A_S), ZPAD_ROW, dtype=np.int64)
        a_of_slot = np.full(A_S, -1, dtype=np.int64)
        b_of_slot = np.full(A_S, -1, dtype=np.int64)
        grp_content = np.zeros(NBG, np.int64)
        for bg in range(NBG):
            pos = bg * BG_AGENTS
            for sl, b in enumerate(members[sh * NBG + bg]):
                lb = bg * BG_BATCHES + sl
                out_map[sh, lb] = b
                lenm1_all[sh, 0, lb] = np.float32(batch_len64[b] - 0.99999)
                n = int(counts[b])
                a_of_slot[pos:pos + n] = np.arange(starts[b], starts[b] + n)
                b_of_slot[pos:pos + n] = lb
                pos += int(padded[b])
            grp_content[bg] = pos - bg * BG_AGENTS
        slots = np.nonzero(a_of_slot >= 0)[0]
        ags = a_of_slot[slots]
        for k in range(BAG):
            idx_flat[k, slots] = x[ags * BAG + k]

        # idx_flat[k] (A_S agents) -> per 1024-agent pair q: [64, 16] -> T
        blocks = (idx_flat.reshape(BAG, NBG // 2, 64, 16)
                  .transpose(0, 1, 3, 2))          # [BAG, 5, 16, 64]
        for k in range(BAG):
            for q in range(NBG // 2):
                off = (k * (NBG // 2) + q) * 64
                # replicated across the 8 Q7 cores' 16-partition groups
                for c in range(8):
                    idx_all[sh, 16 * c:16 * (c + 1),
                            off:off + 64] = blocks[k, q]

        lb_real = b_of_slot[slots]
        rec = recip_f[out_map[sh, lb_real]]
        recip_row = np.zeros(A_S, np.float32)
        recip_row[slots] = rec
        recip_all[sh] = np.broadcast_to(recip_row.astype(bf16), (P, A_S))

        j_of_slot = slots // BG_AGENTS
        w0_al = np.where(j_of_slot >= 8, 128, (j_of_slot // 4) * 64)
        r = lb_real - w0_al
        assert (r >= 0).all() and (r < P).all()
        xs = np.zeros((P, A_S), np.float32)
        xs[r, slots] = rec
        xsel_all[sh] = xs.astype(bf16)

        dbg_slots.append((a_of_slot.copy(), b_of_slot.copy()))
        g_b = b_of_slot[::G]
        for c in range(NCHUNK):
            gl = np.arange(P)
            gb = g_b[c * P + gl]
            v = gb >= 0
            w = gb[v] - W2 * c
            assert (w >= 0).all() and (w < W2).all()
            s2_all[sh][gl[v], c, w] = 1.0

    W0 = np.asarray(W0, np.float32)
    W1 = np.asarray(W1, np.float32)
    wl = np.stack([W0[:, :EDIM].T, W1[:, :EDIM].T])   # [layer, 256k, 256d]
    wr = np.stack([W0[:, EDIM:].T, W1[:, EDIM:].T])

    def tiles(w):  # [2,256,256] -> [128, 2(layer), 2(kt), 2(dt), 128]
        t = w.reshape(2, 2, P, 2, P).transpose(2, 0, 1, 3, 4)
        return np.ascontiguousarray(t).astype(bf16)

    host = {
        "table": np.concatenate(
            [np.asarray(emb_table, np.float32) / 8.0,
             np.zeros((1, EDIM), np.float32)], 0).astype(bf16),
        "wlT": tiles(wl),
        "wrTn": tiles(-wr),
        "wrT": tiles(wr),
        "bias": np.ascontiguousarray(
            np.stack([np.asarray(b0, np.float32), np.asarray(b1, np.float32)])
            .reshape(2, 2, P).transpose(1, 0, 2)[None]  # wrong axis order?
        ),
        "whT": np.ascontiguousarray(
            np.asarray(Wh, np.float32).T.reshape(2, P, N_ACTIONS)
            .transpose(1, 0, 2)).astype(bf16),
        "bh": np.asarray(bh, np.float32).reshape(1, N_ACTIONS).astype(bf16),
        "ones_b": np.ones((1, B_S), bf16),
    }
    # bias layout: [1, 2(layer), 2(dt), 128]
    bias = np.stack([np.asarray(b0, np.float32),
                     np.asarray(b1, np.float32)]).reshape(2, 2, P)
    host["bias"] = bias[None].astype(bf16)

    per_core = []
    for core in range(N_CORES):
        s0 = core * NSUB
        m = dict(host)
        m["idx"] = idx_all[s0:s0 + NSUB]
        m["xsel"] = xsel_all[s0:s0 + NSUB]
        m["recipb"] = recip_all[s0:s0 + NSUB]
        m["s2"] = s2_all[s0:s0 + NSUB]
        m["lenm1"] = lenm1_all[s0:s0 + NSUB]
        per_core.append(m)
    return per_core, out_map, dbg_slots


# ============================================================ device program
def _build_program():
    nc = bacc.Bacc("TRN2")
    bf, f32, i32 = DT.bfloat16, DT.float32, DT.int32

    i16 = DT.int16
    table = nc.dram_tensor("table", [N_WORDS + 1, EDIM], bf,
                           kind="ExternalInput")
    idx_d = nc.dram_tensor("idx", [NSUB, P, BAG * NBG * 32], i16,
                           kind="ExternalInput")
    xsel_d = nc.dram_tensor("xsel", [NSUB, P, A_S], bf, kind="ExternalInput")
    recip_d = nc.dram_tensor("recipb", [NSUB, P, A_S], bf,
                             kind="ExternalInput")
    s2_d = nc.dram_tensor("s2", [NSUB, P, NCHUNK, W2], bf,
                          kind="ExternalInput")
    lenm1_d = nc.dram_tensor("lenm1", [NSUB, 1, B_S], bf,
                             kind="ExternalInput")
    wlT_d = nc.dram_tensor("wlT", [P, 2, 2, 2, P], bf, kind="ExternalInput")
    wrTn_d = nc.dram_tensor("wrTn", [P, 2, 2, 2, P], bf, kind="ExternalInput")
    wrT_d = nc.dram_tensor("wrT", [P, 2, 2, 2, P], bf, kind="ExternalInput")
    bias_d = nc.dram_tensor("bias", [1, 2, 2, P], bf, kind="ExternalInput")
    whT_d = nc.dram_tensor("whT", [P, 2, N_ACTIONS], bf,
                           kind="ExternalInput")
    bh_d = nc.dram_tensor("bh", [1, N_ACTIONS], bf, kind="ExternalInput")
    ones_d = nc.dram_tensor("ones_b", [1, B_S], bf, kind="ExternalInput")
    out_d = nc.dram_tensor("out", [NSUB, B_S, N_ACTIONS], f32,
                           kind="ExternalOutput")
    if _DEBUG:
        dbg_emb0 = nc.dram_tensor("dbg_emb0", [2, P, A_S], bf,
                                  kind="ExternalOutput")
        dbg_emb1 = nc.dram_tensor("dbg_emb1", [2, P, A_S], bf,
                                  kind="ExternalOutput")
        dbg_mT = nc.dram_tensor("dbg_mT", [P, 512], f32,
                                kind="ExternalOutput")
        dbg_x2 = nc.dram_tensor("dbg_x2", [2, P, A_S], bf,
                                kind="ExternalOutput")
        dbg_r = nc.dram_tensor("dbg_r", [P, 2 * B_S], f32,
                               kind="ExternalOutput")
        dbg_h = nc.dram_tensor("dbg_h", [N_ACTIONS, B_S], f32,
                               kind="ExternalOutput")

    with tile.TileContext(nc) as tc, ExitStack() as ctx:
        consts = ctx.enter_context(tc.tile_pool(name="consts", bufs=1))
        wpool = ctx.enter_context(tc.tile_pool(name="wpool", bufs=1))
        gpool = ctx.enter_context(tc.tile_pool(name="gath", bufs=2))
        epool = ctx.enter_context(tc.tile_pool(name="emb", bufs=2))
        e1pool = ctx.enter_context(tc.tile_pool(name="emb1", bufs=1))
        xpool = ctx.enter_context(tc.tile_pool(name="x2p", bufs=1))
        spool = ctx.enter_context(tc.tile_pool(name="small", bufs=2))
        main_ps = ctx.enter_context(
            tc.tile_pool(name="mps", bufs=4, space="PSUM"))
        tp_ps = ctx.enter_context(
            tc.tile_pool(name="tps", bufs=2, space="PSUM"))
        sm_ps = ctx.enter_context(
            tc.tile_pool(name="sps", bufs=2, space="PSUM"))

        ident = consts.tile([P, P], f32, tag="ident", name="ident")
        make_identity(nc, ident[:])
        ident_bf = consts.tile([P, P], bf, tag="ident_bf", name="ident_bf")
        make_identity(nc, ident_bf[:])

        wlT = wpool.tile([P, 2, 2, 2, P], bf, tag="wlT", name="wlT")
        nc.sync.dma_start(wlT[:], wlT_d[:])
        wrTn = wpool.tile([P, 2, 2, 2, P], bf, tag="wrTn", name="wrTn")
        nc.sync.dma_start(wrTn[:], wrTn_d[:])
        wrT = wpool.tile([P, 2, 2, 2, P], bf, tag="wrT", name="wrT")
        nc.sync.dma_start(wrT[:], wrT_d[:])
        bias_sb = wpool.tile([1, 2, 2, P], bf, tag="bias", name="bias")
        nc.sync.dma_start(bias_sb[:], bias_d[:])
        whT = wpool.tile([P, 2, N_ACTIONS], bf, tag="whT", name="whT")
        nc.sync.dma_start(whT[:], whT_d[:])
        bh_sb = wpool.tile([1, N_ACTIONS], bf, tag="bh", name="bh")
        nc.sync.dma_start(bh_sb[:], bh_d[:])
        ones_sb = wpool.tile([1, B_S], bf, tag="ones", name="ones")
        nc.sync.dma_start(ones_sb[:], ones_d[:])

        def tpack(src_aps, dst_ap):
            """PE-transpose [p<=128, w<=128] fp32 APs into one psum bank,
            then one ACT copy (w/ cast) into dst_ap (columns concatenated).
            Each src must have 128 partitions."""
            ps = tp_ps.tile([P, 512], f32, tag="tpack", name="tpack")
            col = 0
            for a in src_aps:
                w = a.shape[-1]
                nc.tensor.transpose(ps[:, col:col + w], a, ident[:])
                col += w
            nc.scalar.activation(dst_ap, ps[:, :col], AF.Copy)

        for s in range(NSUB):
            # ---------------- Phase A: gather/accumulate + transpose
            idx_sb = gpool.tile([P, BAG * NBG * 32], i16, tag="idx",
                                name="idx")
            nc.sync.dma_start(idx_sb[:], idx_d[s])
            xsel = gpool.tile([P, A_S], bf, tag="xsel", name="xsel", bufs=1)
            nc.sync.dma_start(xsel[:], xsel_d[s])
            recipb = gpool.tile([P, A_S], bf, tag="recipb", name="recipb", bufs=1)
            nc.sync.dma_start(recipb[:], recip_d[s])
            s2_sb = gpool.tile([P, NCHUNK, W2], bf, tag="s2", name="s2")
            nc.sync.dma_start(s2_sb[:], s2_d[s])
            lenm1 = gpool.tile([1, B_S], bf, tag="lenm1", name="lenm1")
            nc.sync.dma_start(lenm1[:], lenm1_d[s])

            emb = [None] * NLAYERS
            emb[0] = [epool.tile([P, A_S], bf, tag=f"emb0_{t}", name=f"emb0_{t}")
                      for t in range(2)]
            emb[1] = [e1pool.tile([P, A_S], bf, tag=f"emb1_{t}", name=f"emb1_{t}")
                      for t in range(2)]
            emb[2] = [epool.tile([P, A_S], bf, tag=f"emb0_{t}", name=f"emb0_{t}")
                      for t in range(2)]

            for q in range(NBG // 2):
                slab = gpool.tile([P, BAG, 8, EDIM], bf, tag="slab",
                                  name="slab", bufs=2)
                for k in range(BAG):
                    nc.gpsimd.dma_gather(
                        out_ap=slab[:, k, :, :], in_ap=table[:],
                        idxs_ap=idx_sb[:, (k * (NBG // 2) + q) * 64:
                                       (k * (NBG // 2) + q + 1) * 64],
                        num_idxs=1024, num_idxs_reg=1024, elem_size=EDIM,
                        single_packet=False)
                for t in range(2):
                    for half in range(2):
                        ps = tp_ps.tile([P, 512], f32, tag="tpack",
                                        name="tpa")
                        for c2h in range(4):
                            c2 = half * 4 + c2h
                            for k in range(BAG):
                                nc.tensor.matmul(
                                    ps[:, c2h * P:(c2h + 1) * P],
                                    lhsT=slab[:, k, c2, t * P:(t + 1) * P],
                                    rhs=ident_bf[:],
                                    start=(k == 0), stop=(k == BAG - 1))
                        nc.scalar.activation(
                            emb[0][t][:, (q * 2 + half) * BG_AGENTS:
                                      (q * 2 + half + 1) * BG_AGENTS],
                            ps[:], AF.Copy)

            if _DEBUG and s == 0:
                for t in range(2):
                    nc.sync.dma_start(dbg_emb0[t], emb[0][t][:])

            # ---------------- helpers
            def segsum(src):
                """src = [t0, t1] bf16 [P, A_S] -> m^T psum [P, 512] f32:
                cols 0:256 = batches 0:128, cols 256:512 = batches 128:160
                (on partitions 0:32)."""
                grp = [spool.tile([P, NG], f32, tag=f"grp{t}", name=f"grp{t}",
                                  bufs=1)
                       for t in range(2)]
                for t in range(2):
                    nc.vector.tensor_reduce(
                        grp[t][:],
                        src[t][:].rearrange("p (g e) -> p g e", e=G),
                        axis=mybir.AxisListType.X, op=OP.add)
                gt = spool.tile([P, NCHUNK * EDIM], bf, tag="gt", name="gt")
                for c in range(NCHUNK):
                    tpack([grp[t][:, c * P:(c + 1) * P] for t in range(2)],
                          gt[:, c * EDIM:(c + 1) * EDIM])
                m_ps = sm_ps.tile([P, 512], f32, tag="sps", name="sps")
                for c in range(NCHUNK):
                    r0 = W2 * c if c < 4 else 0
                    dst = (m_ps[r0:r0 + W2, 0:EDIM] if c < 4
                           else m_ps[0:W2, EDIM:2 * EDIM])
                    nc.tensor.matmul(dst, lhsT=s2_sb[:, c, :],
                                     rhs=gt[:, c * EDIM:(c + 1) * EDIM],
                                     start=True, stop=True,
                                     skip_group_check=True,
                                     tile_position=(0, r0))
                return m_ps

            def m_to_sbuf(m_ps):
                mT = spool.tile([P, 512], f32, tag="mT", name="mT")
                nc.scalar.activation(mT[:, 0:EDIM], m_ps[:, 0:EDIM], AF.Copy)
                nc.scalar.activation(mT[0:W2, EDIM:2 * EDIM],
                                     m_ps[0:W2, EDIM:2 * EDIM], AF.Copy)
                return mT

            def m_dimmajor(mT_sb):
                """m^T sbuf -> mdm bf16 [P, 2(dt), B_S] (dim-major m)."""
                ps = sm_ps.tile([P, 512], f32, tag="sps", name="sps")
                for t in range(2):
                    nc.tensor.transpose(ps[:, t * B_S:t * B_S + P],
                                        mT_sb[:, t * P:(t + 1) * P],
                                        ident[:])
                    nc.tensor.transpose(
                        ps[:, t * B_S + P:t * B_S + B_S],
                        mT_sb[0:W2, EDIM + t * P:EDIM + (t + 1) * P],
                        ident[0:W2, 0:W2])
                out = spool.tile([P, 2 * B_S], bf, tag="mdm", name="mdm")
                nc.scalar.activation(out[:], ps[:, 0:2 * B_S], AF.Copy)
                return out

            # ---------------- layers 0, 1
            for i in range(2):
                x2 = [xpool.tile([P, A_S], bf, tag=f"x2_{t}", name=f"x2_{t}")
                      for t in range(2)]
                for t in range(2):
                    nc.vector.tensor_tensor(out=x2[t][:], in0=emb[i][t][:],
                                            in1=recipb[:], op=OP.mult)
                m_ps = segsum(emb[i])
                mT_sb_dbg = m_to_sbuf(m_ps)
                if _DEBUG and s == 0 and i == 0:
                    for t in range(2):
                        nc.sync.dma_start(dbg_x2[t], x2[t][:])
                    nc.sync.dma_start(dbg_mT[:], mT_sb_dbg[:])
                mdm = m_dimmajor(mT_sb_dbg)
                r_ps = sm_ps.tile([P, 512], f32, tag="sps", name="sps")
                for dt in range(2):
                    sl = r_ps[:, dt * B_S:(dt + 1) * B_S]
                    for kt in range(2):
                        nc.tensor.matmul(
                            sl, lhsT=wrT[:, i, kt, dt, :],
                            rhs=mdm[:, kt * B_S:(kt + 1) * B_S],
                            start=(kt == 0), stop=False)
                    nc.tensor.matmul(sl, lhsT=bias_sb[:, i, dt, :],
                                     rhs=lenm1[:], start=False, stop=True)
                r_sb = spool.tile([P, 2 * B_S], f32, tag="r_sb", name="r_sb")
                nc.scalar.activation(r_sb[:], r_ps[:, 0:2 * B_S], AF.Copy)
                if _DEBUG and s == 0 and i == 0:
                    nc.sync.dma_start(dbg_r[:], r_sb[:])
                # R^T at alignments 0 / 64 / 128 -> rt [P, 3, 256] bf16
                rt = spool.tile([P, 3, EDIM], bf, tag="rt", name="rt")
                nc.vector.memset(rt[:], 0.0)
                rt_ps = sm_ps.tile([P, 512], f32, tag="sps", name="sps")
                for dt in range(2):
                    nc.tensor.transpose(
                        rt_ps[:, dt * P:(dt + 1) * P],
                        r_sb[:, dt * B_S:dt * B_S + P], ident[:])
                nc.scalar.activation(rt[:, 0, :], rt_ps[:, 0:EDIM], AF.Copy)
                rt_ps2 = sm_ps.tile([P, 512], f32, tag="sps", name="sps")
                for dt in range(2):
                    nc.tensor.transpose(
                        rt_ps2[0:96, dt * P:(dt + 1) * P],
                        r_sb[:, dt * B_S + 64:dt * B_S + B_S], ident[:])
                    nc.tensor.transpose(
                        rt_ps2[0:W2, EDIM + dt * P:EDIM + dt * P + P],
                        r_sb[:, dt * B_S + P:dt * B_S + B_S], ident[:])
                nc.scalar.activation(rt[0:96, 1, :], rt_ps2[0:96, 0:EDIM],
                                     AF.Copy)
                nc.scalar.activation(rt[0:W2, 2, :],
                                     rt_ps2[0:W2, EDIM:2 * EDIM], AF.Copy)
                # main matmuls
                for j in range(NBG):
                    al = min(j // 4, 2)
                    js = slice(j * BG_AGENTS, (j + 1) * BG_AGENTS)
                    for dt in range(2):
                        ps = main_ps.tile([P, BG_AGENTS], f32, tag="main", name="main")
                        for kt in range(2):
                            nc.tensor.matmul(
                                ps[:], lhsT=wlT[:, i, kt, dt, :],
                                rhs=emb[i][kt][:, js],
                                start=(kt == 0), stop=False)
                        for kt in range(2):
                            nc.tensor.matmul(
                                ps[:], lhsT=wrTn[:, i, kt, dt, :],
                                rhs=x2[kt][:, js], start=False, stop=False)
                        nc.tensor.matmul(
                            ps[:], lhsT=rt[:, al, dt * P:(dt + 1) * P],
                            rhs=xsel[:, js], start=False, stop=True)
                        nc.scalar.activation(emb[i + 1][dt][:, js], ps[:],
                                             AF.Relu)
                if _DEBUG and s == 0 and i == 0:
                    for t in range(2):
                        nc.sync.dma_start(dbg_emb1[t], emb[1][t][:])

            # ---------------- final segsum + head + log_softmax
            m_ps = segsum(emb[2])
            mdm = m_dimmajor(m_to_sbuf(m_ps))
            h_ps = sm_ps.tile([P, 512], f32, tag="sps", name="sps")
            hsl = h_ps[0:N_ACTIONS, 0:B_S]
            for kt in range(2):
                nc.tensor.matmul(hsl, lhsT=whT[:, kt, :],
                                 rhs=mdm[:, kt * B_S:(kt + 1) * B_S],
                                 start=(kt == 0), stop=False)
            nc.tensor.matmul(hsl, lhsT=bh_sb[:], rhs=ones_sb[:],
                             start=False, stop=True)
            h_sb = spool.tile([N_ACTIONS, B_S], f32, tag="h_sb", name="h_sb")
            nc.scalar.activation(h_sb[:], hsl, AF.Copy)
            if _DEBUG and s == 0:
                nc.sync.dma_start(dbg_h[:], h_sb[:])
            lg_ps = sm_ps.tile([P, 512], f32, tag="sps", name="sps")
            nc.tensor.transpose(lg_ps[:, 0:N_ACTIONS], h_sb[:, 0:P],
                                ident[0:N_ACTIONS, 0:N_ACTIONS])
            nc.tensor.transpose(lg_ps[0:W2, N_ACTIONS:2 * N_ACTIONS],
                                h_sb[:, P:B_S],
                                ident[0:N_ACTIONS, 0:N_ACTIONS])
            lg = spool.tile([P, 2 * N_ACTIONS], f32, tag="lg_sb", name="lg_sb")
            nc.scalar.activation(lg[:, 0:N_ACTIONS], lg_ps[:, 0:N_ACTIONS],
                                 AF.Copy)
            nc.scalar.activation(lg[0:W2, N_ACTIONS:2 * N_ACTIONS],
                                 lg_ps[0:W2, N_ACTIONS:2 * N_ACTIONS],
                                 AF.Copy)
            for part in range(2):
                rows = P if part == 0 else B_S - P
                src = lg[0:rows, part * N_ACTIONS:(part + 1) * N_ACTIONS]
                mx = spool.tile([P, 1], f32, tag="mx", name="mx")
                nc.vector.tensor_reduce(mx[0:rows, :], src,
                                        axis=mybir.AxisListType.X,
                                        op=OP.max)
                shv = spool.tile([P, N_ACTIONS], f32, tag="shift", name="shift")
                nc.vector.tensor_tensor(
                    out=shv[0:rows, :], in0=src,
                    in1=mx[0:rows, :].to_broadcast([rows, N_ACTIONS]),
                    op=OP.subtract)
                ex = spool.tile([P, N_ACTIONS], f32, tag="ex", name="ex")
                se = spool.tile([P, 1], f32, tag="se", name="se")
                nc.scalar.activation(ex[0:rows, :], shv[0:rows, :], AF.Exp,
                                     accum_out=se[0:rows, :])
                lse = spool.tile([P, 1], f32, tag="lse", name="lse")
                nc.scalar.activation(lse[0:rows, :], se[0:rows, :], AF.Ln)
                res = spool.tile([P, N_ACTIONS], f32, tag="res", name="res")
                nc.vector.tensor_tensor(
                    out=res[0:rows, :], in0=shv[0:rows, :],
                    in1=lse[0:rows, :].to_broadcast([rows, N_ACTIONS]),
                    op=OP.subtract)
                nc.sync.dma_start(out_d[s, part * P:part * P + rows, :],
                                  res[0:rows, :])
    nc.compile()
    return nc


# ================================================================== kernel
def kernel(**inputs):
    per_core, out_map, _ = _build_host_inputs(
        inputs["x"], inputs["batch_idx"], inputs["batch_len"],
        inputs["emb_table"], inputs["W0"], inputs["b0"], inputs["W1"],
        inputs["b1"], inputs["Wh"], inputs["bh"])

    if "prog" not in _PROGRAM_CACHE:
        _PROGRAM_CACHE["prog"] = _build_program()
    nc = _PROGRAM_CACHE["prog"]

    from concourse.bass_utils import run_bass_kernel_spmd
    res = run_bass_kernel_spmd(nc, per_core, core_ids=list(range(N_CORES)))

    logp = np.zeros((N_BATCHES, N_ACTIONS), np.float32)
    for core in range(N_CORES):
        out = np.asarray(res.results[core]["out"], np.float32)
        for s in range(NSUB):
            sh = core * NSUB + s
            v = out_map[sh] >= 0
            logp[out_map[sh][v]] = out[s][v]
    return (logp,)



# revision 10
# speedup vs baseline: 1.3677x; 1.3677x over previous
"""Trainium2 Bass kernel for nn_Commnet (gnn_message_passing).

kernel(**inputs) takes FULL unsharded numpy inputs, returns (logp [4096,16],)
computed across 8 NeuronCores (SPMD single program; per-core structure is
carried entirely in input tensors).

Sharding: batches bin-packed into 32 sub-shards (4/core); each sub-shard =
10 batch-groups of 16 batch slots padded to exactly 512 agent slots, so every
512-agent matmul block has a static 16-batch selector window -> the program
is identical on all cores.

EmbeddingBag(mean): table cast to bf16 pre-scaled by 1/8 on host (exact);
SWDGE transpose-mode dma_gather (bag-major int16 idx stream wrapped in 16
partitions, replicated across the 8 Q7 cores) fetches word rows directly
dim-major ([128, 2, nwords]), round-robined over 4 SWDGE queues so the
per-512B-descriptor ring drain overlaps; the bag-sum is a DVE group-8
tensor_reduce straight into the emb dim-tiles.

Middle layers: emb' = relu(W_l@emb - W_r@(emb*recip) + R'@sel) where
R' = W_r@m + b (x) (len-0.99999) folds expansion+bias; sel is a banded
one-hot*recip selector (1 extra k-tile per block). m = segment sum via DVE
group-sum(8) -> PE transpose -> banded matmuls into disjoint PSUM windows.
Padded agents stay exactly 0 through all layers.
"""

from contextlib import ExitStack

import numpy as np
import ml_dtypes

import concourse.bass as bass
import concourse.bacc as bacc
import concourse.tile as tile
from concourse import mybir
from concourse.masks import make_identity

N_WORDS = 32000
EDIM = 256
N_AGENTS = 131072
BAG = 8
N_BATCHES = 4096
N_ACTIONS = 16
NLAYERS = 3

N_CORES = 8
NSUB = 4                  # sub-shards per core
NSHARD = N_CORES * NSUB   # 32
NBG = 10                  # batch-groups per sub-shard
BG_BATCHES = 16
BG_AGENTS = 512
B_S = NBG * BG_BATCHES    # 160
A_S = NBG * BG_AGENTS     # 5120
G = 8
NG = A_S // G             # 640
NCHUNK = NG // 128        # 5
W2 = 2 * BG_BATCHES       # 32
P = 128
ZPAD_ROW = N_WORDS
DT = mybir.dt
AF = mybir.ActivationFunctionType
OP = mybir.AluOpType

_PROGRAM_CACHE = {}
_DEBUG = False


# ================================================================ host prep
def _pack_batches(counts):
    padded = ((counts + G - 1) // G) * G
    ngroups = NSHARD * NBG
    cap = np.full(ngroups, BG_AGENTS, dtype=np.int64)
    slots = np.full(ngroups, BG_BATCHES, dtype=np.int64)
    members = [[] for _ in range(ngroups)]
    for b in np.argsort(-padded, kind="stable"):
        ok = (cap >= padded[b]) & (slots > 0)
        if not ok.any():
            raise RuntimeError("bin packing failed")
        g = int(np.argmax(np.where(ok, cap, -1)))
        members[g].append(int(b))
        cap[g] -= padded[b]
        slots[g] -= 1
    return members, padded


def _build_host_inputs(x, batch_idx, batch_len, emb_table, W0, b0, W1, b1,
                       Wh, bh):
    bf16 = ml_dtypes.bfloat16
    x = np.asarray(x, dtype=np.int64)
    batch_idx = np.asarray(batch_idx, dtype=np.int64)
    batch_len64 = np.asarray(batch_len, dtype=np.float64)

    counts = np.bincount(batch_idx, minlength=N_BATCHES).astype(np.int64)
    starts = np.concatenate([[0], np.cumsum(counts)[:-1]])
    members, padded = _pack_batches(counts)

    # transpose-mode dma_gather int16 indices, bag-major stream: stream
    # position i = slot*8 + k; position i lives at [i%16 (+16c), i//16].
    idx_all = np.zeros((NSHARD, P, BAG * A_S // 16), dtype=np.int16)
    xsel_all = np.zeros((NSHARD, P, A_S), dtype=bf16)
    recip_all = np.zeros((NSHARD, P, A_S), dtype=bf16)
    s2_all = np.zeros((NSHARD, P, NCHUNK, W2), dtype=bf16)
    lenm1_all = np.zeros((NSHARD, 1, B_S), dtype=bf16)
    gcnt_all = np.full((NSHARD, 1, NBG // 2), BG_AGENTS * 2, dtype=np.int32)
    out_map = np.full((NSHARD, B_S), -1, dtype=np.int64)
    dbg_slots = []
    recip_f = (1.0 / (batch_len64 - 0.99999)).astype(np.float32)

    for sh in range(NSHARD):
        idx_flat = np.full((BAG, A_S), ZPAD_ROW, dtype=np.int64)
        a_of_slot = np.full(A_S, -1, dtype=np.int64)
        b_of_slot = np.full(A_S, -1, dtype=np.int64)
        grp_content = np.zeros(NBG, np.int64)
        for bg in range(NBG):
            pos = bg * BG_AGENTS
            for sl, b in enumerate(members[sh * NBG + bg]):
                lb = bg * BG_BATCHES + sl
                out_map[sh, lb] = b
                lenm1_all[sh, 0, lb] = np.float32(batch_len64[b] - 0.99999)
                n = int(counts[b])
                a_of_slot[pos:pos + n] = np.arange(starts[b], starts[b] + n)
                b_of_slot[pos:pos + n] = lb
                pos += int(padded[b])
            grp_content[bg] = pos - bg * BG_AGENTS
        slots = np.nonzero(a_of_slot >= 0)[0]
        ags = a_of_slot[slots]
        for k in range(BAG):
            idx_flat[k, slots] = x[ags * BAG + k]

        # bag-major stream: i = slot*8 + k -> wrap [16, i//16], repl x8
        stream = idx_flat.T.reshape(-1)            # [A_S * BAG]
        blk = stream.reshape(-1, 16).T             # [16, A_S * BAG / 16]
        for c in range(8):
            idx_all[sh, 16 * c:16 * (c + 1), :] = blk

        lb_real = b_of_slot[slots]
        rec = recip_f[out_map[sh, lb_real]]
        recip_row = np.zeros(A_S, np.float32)
        recip_row[slots] = rec
        recip_all[sh] = np.broadcast_to(recip_row.astype(bf16), (P, A_S))

        j_of_slot = slots // BG_AGENTS
        w0_al = np.where(j_of_slot >= 8, 128, (j_of_slot // 4) * 64)
        r = lb_real - w0_al
        assert (r >= 0).all() and (r < P).all()
        xs = np.zeros((P, A_S), np.float32)
        xs[r, slots] = rec
        xsel_all[sh] = xs.astype(bf16)

        dbg_slots.append((a_of_slot.copy(), b_of_slot.copy()))
        g_b = b_of_slot[::G]
        for c in range(NCHUNK):
            gl = np.arange(P)
            gb = g_b[c * P + gl]
            v = gb >= 0
            w = gb[v] - W2 * c
            assert (w >= 0).all() and (w < W2).all()
            s2_all[sh][gl[v], c, w] = 1.0

    W0 = np.asarray(W0, np.float32)
    W1 = np.asarray(W1, np.float32)
    wl = np.stack([W0[:, :EDIM].T, W1[:, :EDIM].T])   # [layer, 256k, 256d]
    wr = np.stack([W0[:, EDIM:].T, W1[:, EDIM:].T])

    def tiles(w):  # [2,256,256] -> [128, 2(layer), 2(kt), 2(dt), 128]
        t = w.reshape(2, 2, P, 2, P).transpose(2, 0, 1, 3, 4)
        return np.ascontiguousarray(t).astype(bf16)

    host = {
        "table": np.concatenate(
            [np.asarray(emb_table, np.float32) / 8.0,
             np.zeros((1, EDIM), np.float32)], 0).astype(bf16),
        "wlT": tiles(wl),
        "wrTn": tiles(-wr),
        "wrT": tiles(wr),
        "bias": np.ascontiguousarray(
            np.stack([np.asarray(b0, np.float32), np.asarray(b1, np.float32)])
            .reshape(2, 2, P).transpose(1, 0, 2)[None]  # wrong axis order?
        ),
        "whT": np.ascontiguousarray(
            np.asarray(Wh, np.float32).T.reshape(2, P, N_ACTIONS)
            .transpose(1, 0, 2)).astype(bf16),
        "bh": np.asarray(bh, np.float32).reshape(1, N_ACTIONS).astype(bf16),
        "ones_b": np.ones((1, B_S), bf16),
    }
    # bias layout: [1, 2(layer), 2(dt), 128]
    bias = np.stack([np.asarray(b0, np.float32),
                     np.asarray(b1, np.float32)]).reshape(2, 2, P)
    host["bias"] = bias[None].astype(bf16)

    per_core = []
    for core in range(N_CORES):
        s0 = core * NSUB
        m = dict(host)
        m["idx"] = idx_all[s0:s0 + NSUB]
        m["xsel"] = xsel_all[s0:s0 + NSUB]
        m["recipb"] = recip_all[s0:s0 + NSUB]
        m["s2"] = s2_all[s0:s0 + NSUB]
        m["lenm1"] = lenm1_all[s0:s0 + NSUB]
        per_core.append(m)
    return per_core, out_map, dbg_slots


# ============================================================ device program
def _build_program():
    nc = bacc.Bacc("TRN2", num_swdge_queues=4)
    bf, f32, i32 = DT.bfloat16, DT.float32, DT.int32

    i16 = DT.int16
    table = nc.dram_tensor("table", [N_WORDS + 1, EDIM], bf,
                           kind="ExternalInput")
    idx_d = nc.dram_tensor("idx", [NSUB, P, BAG * A_S // 16], i16,
                           kind="ExternalInput")
    xsel_d = nc.dram_tensor("xsel", [NSUB, P, A_S], bf, kind="ExternalInput")
    recip_d = nc.dram_tensor("recipb", [NSUB, P, A_S], bf,
                             kind="ExternalInput")
    s2_d = nc.dram_tensor("s2", [NSUB, P, NCHUNK, W2], bf,
                          kind="ExternalInput")
    lenm1_d = nc.dram_tensor("lenm1", [NSUB, 1, B_S], bf,
                             kind="ExternalInput")
    wlT_d = nc.dram_tensor("wlT", [P, 2, 2, 2, P], bf, kind="ExternalInput")
    wrTn_d = nc.dram_tensor("wrTn", [P, 2, 2, 2, P], bf, kind="ExternalInput")
    wrT_d = nc.dram_tensor("wrT", [P, 2, 2, 2, P], bf, kind="ExternalInput")
    bias_d = nc.dram_tensor("bias", [1, 2, 2, P], bf, kind="ExternalInput")
    whT_d = nc.dram_tensor("whT", [P, 2, N_ACTIONS], bf,
                           kind="ExternalInput")
    bh_d = nc.dram_tensor("bh", [1, N_ACTIONS], bf, kind="ExternalInput")
    ones_d = nc.dram_tensor("ones_b", [1, B_S], bf, kind="ExternalInput")
    out_d = nc.dram_tensor("out", [NSUB, B_S, N_ACTIONS], f32,
                           kind="ExternalOutput")
    if _DEBUG:
        dbg_emb0 = nc.dram_tensor("dbg_emb0", [2, P, A_S], bf,
                                  kind="ExternalOutput")
        dbg_emb1 = nc.dram_tensor("dbg_emb1", [2, P, A_S], bf,
                                  kind="ExternalOutput")
        dbg_mT = nc.dram_tensor("dbg_mT", [P, 512], f32,
                                kind="ExternalOutput")
        dbg_x2 = nc.dram_tensor("dbg_x2", [2, P, A_S], bf,
                                kind="ExternalOutput")
        dbg_r = nc.dram_tensor("dbg_r", [P, 2 * B_S], f32,
                               kind="ExternalOutput")
        dbg_h = nc.dram_tensor("dbg_h", [N_ACTIONS, B_S], f32,
                               kind="ExternalOutput")

    with tile.TileContext(nc) as tc, ExitStack() as ctx:
        consts = ctx.enter_context(tc.tile_pool(name="consts", bufs=1))
        wpool = ctx.enter_context(tc.tile_pool(name="wpool", bufs=1))
        gpool = ctx.enter_context(tc.tile_pool(name="gath", bufs=2))
        epool = ctx.enter_context(tc.tile_pool(name="emb", bufs=2))
        e1pool = ctx.enter_context(tc.tile_pool(name="emb1", bufs=1))
        xpool = ctx.enter_context(tc.tile_pool(name="x2p", bufs=1))
        spool = ctx.enter_context(tc.tile_pool(name="small", bufs=2))
        main_ps = ctx.enter_context(
            tc.tile_pool(name="mps", bufs=4, space="PSUM"))
        tp_ps = ctx.enter_context(
            tc.tile_pool(name="tps", bufs=2, space="PSUM"))
        sm_ps = ctx.enter_context(
            tc.tile_pool(name="sps", bufs=2, space="PSUM"))

        ident = consts.tile([P, P], f32, tag="ident", name="ident")
        make_identity(nc, ident[:])

        wlT = wpool.tile([P, 2, 2, 2, P], bf, tag="wlT", name="wlT")
        nc.sync.dma_start(wlT[:], wlT_d[:])
        wrTn = wpool.tile([P, 2, 2, 2, P], bf, tag="wrTn", name="wrTn")
        nc.sync.dma_start(wrTn[:], wrTn_d[:])
        wrT = wpool.tile([P, 2, 2, 2, P], bf, tag="wrT", name="wrT")
        nc.sync.dma_start(wrT[:], wrT_d[:])
        bias_sb = wpool.tile([1, 2, 2, P], bf, tag="bias", name="bias")
        nc.sync.dma_start(bias_sb[:], bias_d[:])
        whT = wpool.tile([P, 2, N_ACTIONS], bf, tag="whT", name="whT")
        nc.sync.dma_start(whT[:], whT_d[:])
        bh_sb = wpool.tile([1, N_ACTIONS], bf, tag="bh", name="bh")
        nc.sync.dma_start(bh_sb[:], bh_d[:])
        ones_sb = wpool.tile([1, B_S], bf, tag="ones", name="ones")
        nc.sync.dma_start(ones_sb[:], ones_d[:])

        def tpack(src_aps, dst_ap):
            """PE-transpose [p<=128, w<=128] fp32 APs into one psum bank,
            then one ACT copy (w/ cast) into dst_ap (columns concatenated).
            Each src must have 128 partitions."""
            ps = tp_ps.tile([P, 512], f32, tag="tpack", name="tpack")
            col = 0
            for a in src_aps:
                w = a.shape[-1]
                nc.tensor.transpose(ps[:, col:col + w], a, ident[:])
                col += w
            nc.scalar.activation(dst_ap, ps[:, :col], AF.Copy)

        NOPW = 1024               # words per transpose-gather op
                                  # (4096 wedges the device in transpose mode)
        NGOP = A_S * BAG // NOPW  # 40 gather ops per sub-shard
        for s in range(NSUB):
            # -------- Phase A: transpose-gather + DVE group-8 bag-sum
            idx_sb = gpool.tile([P, BAG * A_S // 16], i16, tag="idx",
                                name="idx")
            nc.sync.dma_start(idx_sb[:], idx_d[s])
            xsel = gpool.tile([P, A_S], bf, tag="xsel", name="xsel", bufs=1)
            nc.sync.dma_start(xsel[:], xsel_d[s])
            recipb = gpool.tile([P, A_S], bf, tag="recipb", name="recipb", bufs=1)
            nc.sync.dma_start(recipb[:], recip_d[s])
            s2_sb = gpool.tile([P, NCHUNK, W2], bf, tag="s2", name="s2")
            nc.sync.dma_start(s2_sb[:], s2_d[s])
            lenm1 = gpool.tile([1, B_S], bf, tag="lenm1", name="lenm1")
            nc.sync.dma_start(lenm1[:], lenm1_d[s])

            emb = [None] * NLAYERS
            emb[0] = [epool.tile([P, A_S], bf, tag=f"emb0_{t}", name=f"emb0_{t}")
                      for t in range(2)]
            emb[1] = [e1pool.tile([P, A_S], bf, tag=f"emb1_{t}", name=f"emb1_{t}")
                      for t in range(2)]
            emb[2] = [epool.tile([P, A_S], bf, tag=f"emb0_{t}", name=f"emb0_{t}")
                      for t in range(2)]

            for q in range(NGOP):
                slab = gpool.tile([P, 2, NOPW], bf, tag="slab",
                                  name="slab", bufs=4)
                nc.gpsimd.dma_gather(
                    out_ap=slab[:], in_ap=table[:],
                    idxs_ap=idx_sb[:, q * (NOPW // 16):
                                   (q + 1) * (NOPW // 16)],
                    num_idxs=NOPW, num_idxs_reg=NOPW, elem_size=EDIM,
                    transpose=True, single_packet=False,
                    queue_num=q % 4)
                na = NOPW // BAG  # 512 agent slots per op
                with nc.allow_low_precision(
                        reason="bag-sum of 8 bf16 rows; ~0.3% rms, "
                               "well under the 2e-2 budget"):
                    for t in range(2):
                        nc.vector.tensor_reduce(
                            emb[0][t][:, q * na:(q + 1) * na],
                            slab[:, t, :].rearrange("p (a g) -> p a g",
                                                    g=BAG),
                            axis=mybir.AxisListType.X, op=OP.add)

            if _DEBUG and s == 0:
                for t in range(2):
                    nc.sync.dma_start(dbg_emb0[t], emb[0][t][:])

            # ---------------- helpers
            def segsum(src):
                """src = [t0, t1] bf16 [P, A_S] -> m^T psum [P, 512] f32:
                cols 0:256 = batches 0:128, cols 256:512 = batches 128:160
                (on partitions 0:32)."""
                grp = [spool.tile([P, NG], f32, tag=f"grp{t}", name=f"grp{t}",
                                  bufs=1)
                       for t in range(2)]
                for t in range(2):
                    nc.vector.tensor_reduce(
                        grp[t][:],
                        src[t][:].rearrange("p (g e) -> p g e", e=G),
                        axis=mybir.AxisListType.X, op=OP.add)
                gt = spool.tile([P, NCHUNK * EDIM], bf, tag="gt", name="gt")
                for c in range(NCHUNK):
                    tpack([grp[t][:, c * P:(c + 1) * P] for t in range(2)],
                          gt[:, c * EDIM:(c + 1) * EDIM])
                m_ps = sm_ps.tile([P, 512], f32, tag="sps", name="sps")
                for c in range(NCHUNK):
                    r0 = W2 * c if c < 4 else 0
                    dst = (m_ps[r0:r0 + W2, 0:EDIM] if c < 4
                           else m_ps[0:W2, EDIM:2 * EDIM])
                    nc.tensor.matmul(dst, lhsT=s2_sb[:, c, :],
                                     rhs=gt[:, c * EDIM:(c + 1) * EDIM],
                                     start=True, stop=True,
                                     skip_group_check=True,
                                     tile_position=(0, r0))
                return m_ps

            def m_to_sbuf(m_ps):
                mT = spool.tile([P, 512], f32, tag="mT", name="mT")
                nc.scalar.activation(mT[:, 0:EDIM], m_ps[:, 0:EDIM], AF.Copy)
                nc.scalar.activation(mT[0:W2, EDIM:2 * EDIM],
                                     m_ps[0:W2, EDIM:2 * EDIM], AF.Copy)
                return mT

            def m_dimmajor(mT_sb):
                """m^T sbuf -> mdm bf16 [P, 2(dt), B_S] (dim-major m)."""
                ps = sm_ps.tile([P, 512], f32, tag="sps", name="sps")
                for t in range(2):
                    nc.tensor.transpose(ps[:, t * B_S:t * B_S + P],
                                        mT_sb[:, t * P:(t + 1) * P],
                                        ident[:])
                    nc.tensor.transpose(
                        ps[:, t * B_S + P:t * B_S + B_S],
                        mT_sb[0:W2, EDIM + t * P:EDIM + (t + 1) * P],
                        ident[0:W2, 0:W2])
                out = spool.tile([P, 2 * B_S], bf, tag="mdm", name="mdm")
                nc.scalar.activation(out[:], ps[:, 0:2 * B_S], AF.Copy)
                return out

            # ---------------- layers 0, 1
            for i in range(2):
                x2 = [xpool.tile([P, A_S], bf, tag=f"x2_{t}", name=f"x2_{t}")
                      for t in range(2)]
                for t in range(2):
                    nc.vector.tensor_tensor(out=x2[t][:], in0=emb[i][t][:],
                                            in1=recipb[:], op=OP.mult)
                m_ps = segsum(emb[i])
                mT_sb_dbg = m_to_sbuf(m_ps)
                if _DEBUG and s == 0 and i == 0:
                    for t in range(2):
                        nc.sync.dma_start(dbg_x2[t], x2[t][:])
                    nc.sync.dma_start(dbg_mT[:], mT_sb_dbg[:])
                mdm = m_dimmajor(mT_sb_dbg)
                r_ps = sm_ps.tile([P, 512], f32, tag="sps", name="sps")
                for dt in range(2):
                    sl = r_ps[:, dt * B_S:(dt + 1) * B_S]
                    for kt in range(2):
                        nc.tensor.matmul(
                            sl, lhsT=wrT[:, i, kt, dt, :],
                            rhs=mdm[:, kt * B_S:(kt + 1) * B_S],
                            start=(kt == 0), stop=False)
                    nc.tensor.matmul(sl, lhsT=bias_sb[:, i, dt, :],
                                     rhs=lenm1[:], start=False, stop=True)
                r_sb = spool.tile([P, 2 * B_S], f32, tag="r_sb", name="r_sb")
                nc.scalar.activation(r_sb[:], r_ps[:, 0:2 * B_S], AF.Copy)
                if _DEBUG and s == 0 and i == 0:
                    nc.sync.dma_start(dbg_r[:], r_sb[:])
                # R^T at alignments 0 / 64 / 128 -> rt [P, 3, 256] bf16
                rt = spool.tile([P, 3, EDIM], bf, tag="rt", name="rt")
                nc.vector.memset(rt[:], 0.0)
                rt_ps = sm_ps.tile([P, 512], f32, tag="sps", name="sps")
                for dt in range(2):
                    nc.tensor.transpose(
                        rt_ps[:, dt * P:(dt + 1) * P],
                        r_sb[:, dt * B_S:dt * B_S + P], ident[:])
                nc.scalar.activation(rt[:, 0, :], rt_ps[:, 0:EDIM], AF.Copy)
                rt_ps2 = sm_ps.tile([P, 512], f32, tag="sps", name="sps")
                for dt in range(2):
                    nc.tensor.transpose(
                        rt_ps2[0:96, dt * P:(dt + 1) * P],
                        r_sb[:, dt * B_S + 64:dt * B_S + B_S], ident[:])
                    nc.tensor.transpose(
                        rt_ps2[0:W2, EDIM + dt * P:EDIM + dt * P + P],
                        r_sb[:, dt * B_S + P:dt * B_S + B_S], ident[:])
                nc.scalar.activation(rt[0:96, 1, :], rt_ps2[0:96, 0:EDIM],
                                     AF.Copy)
                nc.scalar.activation(rt[0:W2, 2, :],
                                     rt_ps2[0:W2, EDIM:2 * EDIM], AF.Copy)
                # main matmuls
                for j in range(NBG):
                    al = min(j // 4, 2)
                    js = slice(j * BG_AGENTS, (j + 1) * BG_AGENTS)
                    for dt in range(2):
                        ps = main_ps.tile([P, BG_AGENTS], f32, tag="main", name="main")
                        for kt in range(2):
                            nc.tensor.matmul(
                                ps[:], lhsT=wlT[:, i, kt, dt, :],
                                rhs=emb[i][kt][:, js],
                                start=(kt == 0), stop=False)
                        for kt in range(2):
                            nc.tensor.matmul(
                                ps[:], lhsT=wrTn[:, i, kt, dt, :],
                                rhs=x2[kt][:, js], start=False, stop=False)
                        nc.tensor.matmul(
                            ps[:], lhsT=rt[:, al, dt * P:(dt + 1) * P],
                            rhs=xsel[:, js], start=False, stop=True)
                        nc.scalar.activation(emb[i + 1][dt][:, js], ps[:],
                                             AF.Relu)
                if _DEBUG and s == 0 and i == 0:
                    for t in range(2):
                        nc.sync.dma_start(dbg_emb1[t], emb[1][t][:])

            # ---------------- final segsum + head + log_softmax
            m_ps = segsum(emb[2])
            mdm = m_dimmajor(m_to_sbuf(m_ps))
            h_ps = sm_ps.tile([P, 512], f32, tag="sps", name="sps")
            hsl = h_ps[0:N_ACTIONS, 0:B_S]
            for kt in range(2):
                nc.tensor.matmul(hsl, lhsT=whT[:, kt, :],
                                 rhs=mdm[:, kt * B_S:(kt + 1) * B_S],
                                 start=(kt == 0), stop=False)
            nc.tensor.matmul(hsl, lhsT=bh_sb[:], rhs=ones_sb[:],
                             start=False, stop=True)
            h_sb = spool.tile([N_ACTIONS, B_S], f32, tag="h_sb", name="h_sb")
            nc.scalar.activation(h_sb[:], hsl, AF.Copy)
            if _DEBUG and s == 0:
                nc.sync.dma_start(dbg_h[:], h_sb[:])
            lg_ps = sm_ps.tile([P, 512], f32, tag="sps", name="sps")
            nc.tensor.transpose(lg_ps[:, 0:N_ACTIONS], h_sb[:, 0:P],
                                ident[0:N_ACTIONS, 0:N_ACTIONS])
            nc.tensor.transpose(lg_ps[0:W2, N_ACTIONS:2 * N_ACTIONS],
                                h_sb[:, P:B_S],
                                ident[0:N_ACTIONS, 0:N_ACTIONS])
            lg = spool.tile([P, 2 * N_ACTIONS], f32, tag="lg_sb", name="lg_sb")
            nc.scalar.activation(lg[:, 0:N_ACTIONS], lg_ps[:, 0:N_ACTIONS],
                                 AF.Copy)
            nc.scalar.activation(lg[0:W2, N_ACTIONS:2 * N_ACTIONS],
                                 lg_ps[0:W2, N_ACTIONS:2 * N_ACTIONS],
                                 AF.Copy)
            for part in range(2):
                rows = P if part == 0 else B_S - P
                src = lg[0:rows, part * N_ACTIONS:(part + 1) * N_ACTIONS]
                mx = spool.tile([P, 1], f32, tag="mx", name="mx")
                nc.vector.tensor_reduce(mx[0:rows, :], src,
                                        axis=mybir.AxisListType.X,
                                        op=OP.max)
                shv = spool.tile([P, N_ACTIONS], f32, tag="shift", name="shift")
                nc.vector.tensor_tensor(
                    out=shv[0:rows, :], in0=src,
                    in1=mx[0:rows, :].to_broadcast([rows, N_ACTIONS]),
                    op=OP.subtract)
                ex = spool.tile([P, N_ACTIONS], f32, tag="ex", name="ex")
                se = spool.tile([P, 1], f32, tag="se", name="se")
                nc.scalar.activation(ex[0:rows, :], shv[0:rows, :], AF.Exp,
                                     accum_out=se[0:rows, :])
                lse = spool.tile([P, 1], f32, tag="lse", name="lse")
                nc.scalar.activation(lse[0:rows, :], se[0:rows, :], AF.Ln)
                res = spool.tile([P, N_ACTIONS], f32, tag="res", name="res")
                nc.vector.tensor_tensor(
                    out=res[0:rows, :], in0=shv[0:rows, :],
                    in1=lse[0:rows, :].to_broadcast([rows, N_ACTIONS]),
                    op=OP.subtract)
                nc.sync.dma_start(out_d[s, part * P:part * P + rows, :],
                                  res[0:rows, :])
    nc.compile()
    return nc


# ================================================================== kernel
def kernel(**inputs):
    per_core, out_map, _ = _build_host_inputs(
        inputs["x"], inputs["batch_idx"], inputs["batch_len"],
        inputs["emb_table"], inputs["W0"], inputs["b0"], inputs["W1"],
        inputs["b1"], inputs["Wh"], inputs["bh"])

    if "prog" not in _PROGRAM_CACHE:
        _PROGRAM_CACHE["prog"] = _build_program()
    nc = _PROGRAM_CACHE["prog"]

    from concourse.bass_utils import run_bass_kernel_spmd
    res = run_bass_kernel_spmd(nc, per_core, core_ids=list(range(N_CORES)))

    logp = np.zeros((N_BATCHES, N_ACTIONS), np.float32)
    for core in range(N_CORES):
        out = np.asarray(res.results[core]["out"], np.float32)
        for s in range(NSUB):
            sh = core * NSUB + s
            v = out_map[sh] >= 0
            logp[out_map[sh][v]] = out[s][v]
    return (logp,)



# revision 17
# speedup vs baseline: 1.5147x; 1.1075x over previous
"""Trainium2 Bass kernel for nn_Commnet (gnn_message_passing).

kernel(**inputs) takes FULL unsharded numpy inputs, returns (logp [4096,16],)
computed across 8 NeuronCores (SPMD single program; per-core structure is
carried entirely in input tensors).

Sharding: batches bin-packed into 32 sub-shards (4/core); each sub-shard =
10 batch-groups of 16 batch slots padded to exactly 512 agent slots, so every
512-agent matmul block has a static 16-batch selector window -> the program
is identical on all cores.

EmbeddingBag(mean): table cast to bf16 pre-scaled by 1/8 on host (exact);
SWDGE transpose-mode dma_gather (bag-major int16 idx stream wrapped in 16
partitions, replicated across the 8 Q7 cores) fetches word rows directly
dim-major ([128, 2, nwords]), round-robined over 4 SWDGE queues so the
per-512B-descriptor ring drain overlaps; the bag-sum is a DVE group-8
tensor_reduce straight into the emb dim-tiles.

Middle layers: emb' = relu(W_l@emb - W_r@(emb*recip) + R'@sel) where
R' = W_r@m + b (x) (len-0.99999) folds expansion+bias; sel is a banded
one-hot*recip selector (1 extra k-tile per block). m = segment sum via DVE
group-sum(8) -> PE transpose -> banded matmuls into disjoint PSUM windows.
Padded agents stay exactly 0 through all layers.
"""

from contextlib import ExitStack

import numpy as np
import ml_dtypes

import concourse.bass as bass
import concourse.bacc as bacc
import concourse.tile as tile
from concourse import mybir
from concourse.masks import make_identity

N_WORDS = 32000
EDIM = 256
N_AGENTS = 131072
BAG = 8
N_BATCHES = 4096
N_ACTIONS = 16
NLAYERS = 3

N_CORES = 8
NSUB = 4                  # sub-shards per core
NSHARD = N_CORES * NSUB   # 32
NBG = 10                  # batch-groups per sub-shard
BG_BATCHES = 16
BG_AGENTS = 512
B_S = NBG * BG_BATCHES    # 160
A_S = NBG * BG_AGENTS     # 5120
G = 8
NG = A_S // G             # 640
NCHUNK = NG // 128        # 5
W2 = 2 * BG_BATCHES       # 32
P = 128
ZPAD_ROW = N_WORDS
DT = mybir.dt
AF = mybir.ActivationFunctionType
OP = mybir.AluOpType

_PROGRAM_CACHE = {}
_DEBUG = False


# ================================================================ host prep
def _pack_batches(counts):
    padded = ((counts + G - 1) // G) * G
    ngroups = NSHARD * NBG
    cap = np.full(ngroups, BG_AGENTS, dtype=np.int64)
    slots = np.full(ngroups, BG_BATCHES, dtype=np.int64)
    members = [[] for _ in range(ngroups)]
    for b in np.argsort(-padded, kind="stable"):
        ok = (cap >= padded[b]) & (slots > 0)
        if not ok.any():
            raise RuntimeError("bin packing failed")
        g = int(np.argmax(np.where(ok, cap, -1)))
        members[g].append(int(b))
        cap[g] -= padded[b]
        slots[g] -= 1
    return members, padded


def _build_host_inputs(x, batch_idx, batch_len, emb_table, W0, b0, W1, b1,
                       Wh, bh):
    bf16 = ml_dtypes.bfloat16
    x = np.asarray(x, dtype=np.int64)
    batch_idx = np.asarray(batch_idx, dtype=np.int64)
    batch_len64 = np.asarray(batch_len, dtype=np.float64)

    counts = np.bincount(batch_idx, minlength=N_BATCHES).astype(np.int64)
    starts = np.concatenate([[0], np.cumsum(counts)[:-1]])
    members, padded = _pack_batches(counts)

    # transpose-mode dma_gather int16 indices, bag-major stream: stream
    # position i = slot*8 + k; position i lives at [i%16 (+16c), i//16].
    idx_all = np.zeros((NSHARD, P, BAG * A_S // 16), dtype=np.int16)
    xsel_all = np.zeros((NSHARD, P, A_S), dtype=bf16)
    recip_all = np.zeros((NSHARD, P, A_S), dtype=bf16)
    s2_all = np.zeros((NSHARD, P, NCHUNK, W2), dtype=bf16)
    lenm1_all = np.zeros((NSHARD, 1, B_S), dtype=bf16)
    gcnt_all = np.full((NSHARD, 1, NBG // 2), BG_AGENTS * 2, dtype=np.int32)
    out_map = np.full((NSHARD, B_S), -1, dtype=np.int64)
    dbg_slots = []
    recip_f = (1.0 / (batch_len64 - 0.99999)).astype(np.float32)

    for sh in range(NSHARD):
        idx_flat = np.full((BAG, A_S), ZPAD_ROW, dtype=np.int64)
        a_of_slot = np.full(A_S, -1, dtype=np.int64)
        b_of_slot = np.full(A_S, -1, dtype=np.int64)
        grp_content = np.zeros(NBG, np.int64)
        for bg in range(NBG):
            pos = bg * BG_AGENTS
            for sl, b in enumerate(members[sh * NBG + bg]):
                lb = bg * BG_BATCHES + sl
                out_map[sh, lb] = b
                lenm1_all[sh, 0, lb] = np.float32(batch_len64[b] - 0.99999)
                n = int(counts[b])
                a_of_slot[pos:pos + n] = np.arange(starts[b], starts[b] + n)
                b_of_slot[pos:pos + n] = lb
                pos += int(padded[b])
            grp_content[bg] = pos - bg * BG_AGENTS
        slots = np.nonzero(a_of_slot >= 0)[0]
        ags = a_of_slot[slots]
        for k in range(BAG):
            idx_flat[k, slots] = x[ags * BAG + k]

        # bag-major stream: i = slot*8 + k -> wrap [16, i//16], repl x8
        stream = idx_flat.T.reshape(-1)            # [A_S * BAG]
        blk = stream.reshape(-1, 16).T             # [16, A_S * BAG / 16]
        for c in range(8):
            idx_all[sh, 16 * c:16 * (c + 1), :] = blk

        lb_real = b_of_slot[slots]
        rec = recip_f[out_map[sh, lb_real]]
        recip_row = np.zeros(A_S, np.float32)
        recip_row[slots] = rec
        recip_all[sh] = np.broadcast_to(recip_row.astype(bf16), (P, A_S))

        j_of_slot = slots // BG_AGENTS
        w0_al = np.where(j_of_slot >= 8, 128, (j_of_slot // 4) * 64)
        r = lb_real - w0_al
        assert (r >= 0).all() and (r < P).all()
        xs = np.zeros((P, A_S), np.float32)
        xs[r, slots] = rec
        xsel_all[sh] = xs.astype(bf16)

        dbg_slots.append((a_of_slot.copy(), b_of_slot.copy()))
        g_b = b_of_slot[::G]
        for c in range(NCHUNK):
            gl = np.arange(P)
            gb = g_b[c * P + gl]
            v = gb >= 0
            w = gb[v] - W2 * c
            assert (w >= 0).all() and (w < W2).all()
            s2_all[sh][gl[v], c, w] = 1.0

    W0 = np.asarray(W0, np.float32)
    W1 = np.asarray(W1, np.float32)
    wl = np.stack([W0[:, :EDIM].T, W1[:, :EDIM].T])   # [layer, 256k, 256d]
    wr = np.stack([W0[:, EDIM:].T, W1[:, EDIM:].T])

    def tiles(w):  # [2,256,256] -> [128, 2(layer), 2(kt), 2(dt), 128]
        t = w.reshape(2, 2, P, 2, P).transpose(2, 0, 1, 3, 4)
        return np.ascontiguousarray(t).astype(bf16)

    host = {
        "table": np.concatenate(
            [np.asarray(emb_table, np.float32) / 8.0,
             np.zeros((1, EDIM), np.float32)], 0).astype(bf16),
        "wlT": tiles(wl),
        "wrTn": tiles(-wr),
        "wrT": tiles(wr),
        "bias": np.ascontiguousarray(
            np.stack([np.asarray(b0, np.float32), np.asarray(b1, np.float32)])
            .reshape(2, 2, P).transpose(1, 0, 2)[None]  # wrong axis order?
        ),
        "whT": np.ascontiguousarray(
            np.asarray(Wh, np.float32).T.reshape(2, P, N_ACTIONS)
            .transpose(1, 0, 2)).astype(bf16),
        "bh": np.asarray(bh, np.float32).reshape(1, N_ACTIONS).astype(bf16),
        "ones_b": np.ones((1, B_S), bf16),
    }
    # bias layout: [1, 2(layer), 2(dt), 128]
    bias = np.stack([np.asarray(b0, np.float32),
                     np.asarray(b1, np.float32)]).reshape(2, 2, P)
    host["bias"] = bias[None].astype(bf16)

    per_core = []
    for core in range(N_CORES):
        s0 = core * NSUB
        m = dict(host)
        m["idx"] = idx_all[s0:s0 + NSUB]
        m["xsel"] = xsel_all[s0:s0 + NSUB]
        m["recipb"] = recip_all[s0:s0 + NSUB]
        m["s2"] = s2_all[s0:s0 + NSUB]
        m["lenm1"] = lenm1_all[s0:s0 + NSUB]
        per_core.append(m)
    return per_core, out_map, dbg_slots


# ============================================================ device program
def _build_program():
    nc = bacc.Bacc("TRN2", num_swdge_queues=4)
    bf, f32, i32 = DT.bfloat16, DT.float32, DT.int32

    i16 = DT.int16
    table = nc.dram_tensor("table", [N_WORDS + 1, EDIM], bf,
                           kind="ExternalInput")
    idx_d = nc.dram_tensor("idx", [NSUB, P, BAG * A_S // 16], i16,
                           kind="ExternalInput")
    xsel_d = nc.dram_tensor("xsel", [NSUB, P, A_S], bf, kind="ExternalInput")
    recip_d = nc.dram_tensor("recipb", [NSUB, P, A_S], bf,
                             kind="ExternalInput")
    s2_d = nc.dram_tensor("s2", [NSUB, P, NCHUNK, W2], bf,
                          kind="ExternalInput")
    lenm1_d = nc.dram_tensor("lenm1", [NSUB, 1, B_S], bf,
                             kind="ExternalInput")
    wlT_d = nc.dram_tensor("wlT", [P, 2, 2, 2, P], bf, kind="ExternalInput")
    wrTn_d = nc.dram_tensor("wrTn", [P, 2, 2, 2, P], bf, kind="ExternalInput")
    wrT_d = nc.dram_tensor("wrT", [P, 2, 2, 2, P], bf, kind="ExternalInput")
    bias_d = nc.dram_tensor("bias", [1, 2, 2, P], bf, kind="ExternalInput")
    whT_d = nc.dram_tensor("whT", [P, 2, N_ACTIONS], bf,
                           kind="ExternalInput")
    bh_d = nc.dram_tensor("bh", [1, N_ACTIONS], bf, kind="ExternalInput")
    ones_d = nc.dram_tensor("ones_b", [1, B_S], bf, kind="ExternalInput")
    out_d = nc.dram_tensor("out", [NSUB, B_S, N_ACTIONS], f32,
                           kind="ExternalOutput")
    if _DEBUG:
        dbg_emb0 = nc.dram_tensor("dbg_emb0", [2, P, A_S], bf,
                                  kind="ExternalOutput")
        dbg_emb1 = nc.dram_tensor("dbg_emb1", [2, P, A_S], bf,
                                  kind="ExternalOutput")
        dbg_mT = nc.dram_tensor("dbg_mT", [P, 512], f32,
                                kind="ExternalOutput")
        dbg_x2 = nc.dram_tensor("dbg_x2", [2, P, A_S], bf,
                                kind="ExternalOutput")
        dbg_r = nc.dram_tensor("dbg_r", [P, 2 * B_S], f32,
                               kind="ExternalOutput")
        dbg_h = nc.dram_tensor("dbg_h", [N_ACTIONS, B_S], f32,
                               kind="ExternalOutput")

    with tile.TileContext(nc) as tc, ExitStack() as ctx:
        consts = ctx.enter_context(tc.tile_pool(name="consts", bufs=1))
        wpool = ctx.enter_context(tc.tile_pool(name="wpool", bufs=1))
        gpool = ctx.enter_context(tc.tile_pool(name="gath", bufs=2))
        e0pool = ctx.enter_context(tc.tile_pool(name="emb0", bufs=2))
        e1pool = ctx.enter_context(tc.tile_pool(name="emb1", bufs=2))
        e2pool = ctx.enter_context(tc.tile_pool(name="emb2", bufs=1))
        xpool = ctx.enter_context(tc.tile_pool(name="x2p", bufs=1))
        spool = ctx.enter_context(tc.tile_pool(name="small", bufs=2))
        main_ps = ctx.enter_context(
            tc.tile_pool(name="mps", bufs=4, space="PSUM"))
        tp_ps = ctx.enter_context(
            tc.tile_pool(name="tps", bufs=2, space="PSUM"))
        sm_ps = ctx.enter_context(
            tc.tile_pool(name="sps", bufs=2, space="PSUM"))

        ident = consts.tile([P, P], f32, tag="ident", name="ident")
        make_identity(nc, ident[:])

        wlT = wpool.tile([P, 2, 2, 2, P], bf, tag="wlT", name="wlT")
        nc.sync.dma_start(wlT[:], wlT_d[:])
        wrTn = wpool.tile([P, 2, 2, 2, P], bf, tag="wrTn", name="wrTn")
        nc.sync.dma_start(wrTn[:], wrTn_d[:])
        wrT = wpool.tile([P, 2, 2, 2, P], bf, tag="wrT", name="wrT")
        nc.sync.dma_start(wrT[:], wrT_d[:])
        bias_sb = wpool.tile([1, 2, 2, P], bf, tag="bias", name="bias")
        nc.sync.dma_start(bias_sb[:], bias_d[:])
        whT = wpool.tile([P, 2, N_ACTIONS], bf, tag="whT", name="whT")
        nc.sync.dma_start(whT[:], whT_d[:])
        bh_sb = wpool.tile([1, N_ACTIONS], bf, tag="bh", name="bh")
        nc.sync.dma_start(bh_sb[:], bh_d[:])
        ones_sb = wpool.tile([1, B_S], bf, tag="ones", name="ones")
        nc.sync.dma_start(ones_sb[:], ones_d[:])

        def tpack(src_aps, dst_ap):
            """PE-transpose [p<=128, w<=128] fp32 APs into one psum bank,
            then one ACT copy (w/ cast) into dst_ap (columns concatenated).
            Each src must have 128 partitions."""
            ps = tp_ps.tile([P, 512], f32, tag="tpack", name="tpack")
            col = 0
            for a in src_aps:
                w = a.shape[-1]
                nc.tensor.transpose(ps[:, col:col + w], a, ident[:])
                col += w
            nc.scalar.activation(dst_ap, ps[:, :col], AF.Copy)

        NOPW = 1024               # words per transpose-gather op
                                  # (4096 wedges the device in transpose mode)
        NGOP = A_S * BAG // NOPW  # 40 gather ops per sub-shard
        for s in range(NSUB):
            # -------- Phase A: transpose-gather + DVE group-8 bag-sum
            idx_sb = gpool.tile([P, BAG * A_S // 16], i16, tag="idx",
                                name="idx")
            nc.sync.dma_start(idx_sb[:], idx_d[s])
            xsel = gpool.tile([P, A_S], bf, tag="xsel", name="xsel", bufs=1)
            nc.sync.dma_start(xsel[:], xsel_d[s])
            recipb = gpool.tile([P, A_S], bf, tag="recipb", name="recipb", bufs=1)
            nc.sync.dma_start(recipb[:], recip_d[s])
            s2_sb = gpool.tile([P, NCHUNK, W2], bf, tag="s2", name="s2")
            nc.sync.dma_start(s2_sb[:], s2_d[s])
            lenm1 = gpool.tile([1, B_S], bf, tag="lenm1", name="lenm1")
            nc.sync.dma_start(lenm1[:], lenm1_d[s])

            emb = [None] * NLAYERS
            emb[0] = e0pool.tile([P, 2, A_S], bf, tag="emb0", name="emb0")
            emb[1] = e1pool.tile([P, 2, A_S], bf, tag="emb1", name="emb1")
            emb[2] = e2pool.tile([P, 2, A_S], bf, tag="emb2", name="emb2")

            for q in range(NGOP):
                slab = gpool.tile([P, 2, NOPW], bf, tag="slab",
                                  name="slab", bufs=4)
                nc.gpsimd.dma_gather(
                    out_ap=slab[:], in_ap=table[:],
                    idxs_ap=idx_sb[:, q * (NOPW // 16):
                                   (q + 1) * (NOPW // 16)],
                    num_idxs=NOPW, num_idxs_reg=NOPW, elem_size=EDIM,
                    transpose=True, single_packet=False,
                    queue_num=q % 4)
                na = NOPW // BAG  # 128 agent slots per op
                with nc.allow_low_precision(
                        reason="bag-sum of 8 bf16 rows; ~0.3% rms, "
                               "well under the 2e-2 budget"):
                    nc.vector.tensor_reduce(
                        emb[0][:, :, q * na:(q + 1) * na],
                        slab[:].rearrange("p t (a g) -> p t a g", g=BAG),
                        axis=mybir.AxisListType.X, op=OP.add)

            if _DEBUG and s == 0:
                for t in range(2):
                    nc.sync.dma_start(dbg_emb0[t], emb[0][:, t, :])

            # ---------------- helpers
            def segsum(src):
                """src = bf16 [P, 2, A_S] -> m^T psum [P, 512] f32:
                cols 0:256 = batches 0:128, cols 256:512 = batches 128:160
                (on partitions 0:32)."""
                grp = spool.tile([P, 2, NG], f32, tag="grp", name="grp",
                                 bufs=1)
                nc.vector.tensor_reduce(
                    grp[:],
                    src[:].rearrange("p t (g e) -> p t g e", e=G),
                    axis=mybir.AxisListType.X, op=OP.add)
                gt = spool.tile([P, NCHUNK * EDIM], bf, tag="gt", name="gt")
                for c in range(NCHUNK):
                    tpack([grp[:, t, c * P:(c + 1) * P] for t in range(2)],
                          gt[:, c * EDIM:(c + 1) * EDIM])
                m_ps = sm_ps.tile([P, 512], f32, tag="sps", name="sps")
                for c in range(NCHUNK):
                    r0 = W2 * c if c < 4 else 0
                    dst = (m_ps[r0:r0 + W2, 0:EDIM] if c < 4
                           else m_ps[0:W2, EDIM:2 * EDIM])
                    nc.tensor.matmul(dst, lhsT=s2_sb[:, c, :],
                                     rhs=gt[:, c * EDIM:(c + 1) * EDIM],
                                     start=True, stop=True,
                                     skip_group_check=True,
                                     tile_position=(0, r0))
                return m_ps

            def m_to_sbuf(m_ps):
                mT = spool.tile([P, 512], f32, tag="mT", name="mT")
                nc.scalar.activation(mT[:, 0:EDIM], m_ps[:, 0:EDIM], AF.Copy)
                nc.scalar.activation(mT[0:W2, EDIM:2 * EDIM],
                                     m_ps[0:W2, EDIM:2 * EDIM], AF.Copy)
                return mT

            def m_dimmajor(mT_sb):
                """m^T sbuf -> mdm bf16 [P, 2(dt), B_S] (dim-major m)."""
                ps = sm_ps.tile([P, 512], f32, tag="sps", name="sps")
                for t in range(2):
                    nc.tensor.transpose(ps[:, t * B_S:t * B_S + P],
                                        mT_sb[:, t * P:(t + 1) * P],
                                        ident[:])
                    nc.tensor.transpose(
                        ps[:, t * B_S + P:t * B_S + B_S],
                        mT_sb[0:W2, EDIM + t * P:EDIM + (t + 1) * P],
                        ident[0:W2, 0:W2])
                out = spool.tile([P, 2 * B_S], bf, tag="mdm", name="mdm")
                nc.scalar.activation(out[:], ps[:, 0:2 * B_S], AF.Copy)
                return out

            # ---------------- layers 0, 1
            for i in range(2):
                x2 = xpool.tile([P, 2, A_S], bf, tag="x2", name="x2")
                for t in range(2):
                    nc.vector.tensor_tensor(out=x2[:, t, :],
                                            in0=emb[i][:, t, :],
                                            in1=recipb[:], op=OP.mult)
                m_ps = segsum(emb[i])
                mT_sb_dbg = m_to_sbuf(m_ps)
                if _DEBUG and s == 0 and i == 0:
                    for t in range(2):
                        nc.sync.dma_start(dbg_x2[t], x2[:, t, :])
                    nc.sync.dma_start(dbg_mT[:], mT_sb_dbg[:])
                mdm = m_dimmajor(mT_sb_dbg)
                r_ps = sm_ps.tile([P, 512], f32, tag="sps", name="sps")
                for dt in range(2):
                    sl = r_ps[:, dt * B_S:(dt + 1) * B_S]
                    for kt in range(2):
                        nc.tensor.matmul(
                            sl, lhsT=wrT[:, i, kt, dt, :],
                            rhs=mdm[:, kt * B_S:(kt + 1) * B_S],
                            start=(kt == 0), stop=False)
                    nc.tensor.matmul(sl, lhsT=bias_sb[:, i, dt, :],
                                     rhs=lenm1[:], start=False, stop=True)
                r_sb = spool.tile([P, 2 * B_S], f32, tag="r_sb", name="r_sb")
                nc.scalar.activation(r_sb[:], r_ps[:, 0:2 * B_S], AF.Copy)
                if _DEBUG and s == 0 and i == 0:
                    nc.sync.dma_start(dbg_r[:], r_sb[:])
                # R^T at alignments 0 / 64 / 128 -> rt [P, 3, 256] bf16
                rt = spool.tile([P, 3, EDIM], bf, tag="rt", name="rt")
                nc.vector.memset(rt[:], 0.0)
                rt_ps = sm_ps.tile([P, 512], f32, tag="sps", name="sps")
                for dt in range(2):
                    nc.tensor.transpose(
                        rt_ps[:, dt * P:(dt + 1) * P],
                        r_sb[:, dt * B_S:dt * B_S + P], ident[:])
                nc.scalar.activation(rt[:, 0, :], rt_ps[:, 0:EDIM], AF.Copy)
                rt_ps2 = sm_ps.tile([P, 512], f32, tag="sps", name="sps")
                for dt in range(2):
                    nc.tensor.transpose(
                        rt_ps2[0:96, dt * P:(dt + 1) * P],
                        r_sb[:, dt * B_S + 64:dt * B_S + B_S], ident[:])
                    nc.tensor.transpose(
                        rt_ps2[0:W2, EDIM + dt * P:EDIM + dt * P + P],
                        r_sb[:, dt * B_S + P:dt * B_S + B_S], ident[:])
                nc.scalar.activation(rt[0:96, 1, :], rt_ps2[0:96, 0:EDIM],
                                     AF.Copy)
                nc.scalar.activation(rt[0:W2, 2, :],
                                     rt_ps2[0:W2, EDIM:2 * EDIM], AF.Copy)
                # main matmuls
                for j in range(NBG):
                    al = min(j // 4, 2)
                    js = slice(j * BG_AGENTS, (j + 1) * BG_AGENTS)
                    for dt in range(2):
                        ps = main_ps.tile([P, BG_AGENTS], f32, tag="main", name="main")
                        for kt in range(2):
                            nc.tensor.matmul(
                                ps[:], lhsT=wlT[:, i, kt, dt, :],
                                rhs=emb[i][:, kt, js],
                                start=(kt == 0), stop=False)
                        for kt in range(2):
                            nc.tensor.matmul(
                                ps[:], lhsT=wrTn[:, i, kt, dt, :],
                                rhs=x2[:, kt, js], start=False, stop=False)
                        nc.tensor.matmul(
                            ps[:], lhsT=rt[:, al, dt * P:(dt + 1) * P],
                            rhs=xsel[:, js], start=False, stop=True)
                        nc.scalar.activation(emb[i + 1][:, dt, js], ps[:],
                                             AF.Relu)
                if _DEBUG and s == 0 and i == 0:
                    for t in range(2):
                        nc.sync.dma_start(dbg_emb1[t], emb[1][:, t, :])

            # ---------------- final segsum + head + log_softmax
            m_ps = segsum(emb[2])
            mdm = m_dimmajor(m_to_sbuf(m_ps))
            h_ps = sm_ps.tile([P, 512], f32, tag="sps", name="sps")
            hsl = h_ps[0:N_ACTIONS, 0:B_S]
            for kt in range(2):
                nc.tensor.matmul(hsl, lhsT=whT[:, kt, :],
                                 rhs=mdm[:, kt * B_S:(kt + 1) * B_S],
                                 start=(kt == 0), stop=False)
            nc.tensor.matmul(hsl, lhsT=bh_sb[:], rhs=ones_sb[:],
                             start=False, stop=True)
            h_sb = spool.tile([N_ACTIONS, B_S], f32, tag="h_sb", name="h_sb")
            nc.scalar.activation(h_sb[:], hsl, AF.Copy)
            if _DEBUG and s == 0:
                nc.sync.dma_start(dbg_h[:], h_sb[:])
            lg_ps = sm_ps.tile([P, 512], f32, tag="sps", name="sps")
            nc.tensor.transpose(lg_ps[:, 0:N_ACTIONS], h_sb[:, 0:P],
                                ident[0:N_ACTIONS, 0:N_ACTIONS])
            nc.tensor.transpose(lg_ps[0:W2, N_ACTIONS:2 * N_ACTIONS],
                                h_sb[:, P:B_S],
                                ident[0:N_ACTIONS, 0:N_ACTIONS])
            lg = spool.tile([P, 2 * N_ACTIONS], f32, tag="lg_sb", name="lg_sb")
            nc.scalar.activation(lg[:, 0:N_ACTIONS], lg_ps[:, 0:N_ACTIONS],
                                 AF.Copy)
            nc.scalar.activation(lg[0:W2, N_ACTIONS:2 * N_ACTIONS],
                                 lg_ps[0:W2, N_ACTIONS:2 * N_ACTIONS],
                                 AF.Copy)
            for part in range(2):
                rows = P if part == 0 else B_S - P
                src = lg[0:rows, part * N_ACTIONS:(part + 1) * N_ACTIONS]
                mx = spool.tile([P, 1], f32, tag="mx", name="mx")
                nc.vector.tensor_reduce(mx[0:rows, :], src,
                                        axis=mybir.AxisListType.X,
                                        op=OP.max)
                shv = spool.tile([P, N_ACTIONS], f32, tag="shift", name="shift")
                nc.vector.tensor_tensor(
                    out=shv[0:rows, :], in0=src,
                    in1=mx[0:rows, :].to_broadcast([rows, N_ACTIONS]),
                    op=OP.subtract)
                ex = spool.tile([P, N_ACTIONS], f32, tag="ex", name="ex")
                se = spool.tile([P, 1], f32, tag="se", name="se")
                nc.scalar.activation(ex[0:rows, :], shv[0:rows, :], AF.Exp,
                                     accum_out=se[0:rows, :])
                lse = spool.tile([P, 1], f32, tag="lse", name="lse")
                nc.scalar.activation(lse[0:rows, :], se[0:rows, :], AF.Ln)
                res = spool.tile([P, N_ACTIONS], f32, tag="res", name="res")
                nc.vector.tensor_tensor(
                    out=res[0:rows, :], in0=shv[0:rows, :],
                    in1=lse[0:rows, :].to_broadcast([rows, N_ACTIONS]),
                    op=OP.subtract)
                nc.sync.dma_start(out_d[s, part * P:part * P + rows, :],
                                  res[0:rows, :])
    nc.compile()
    return nc


# ================================================================== kernel
def kernel(**inputs):
    per_core, out_map, _ = _build_host_inputs(
        inputs["x"], inputs["batch_idx"], inputs["batch_len"],
        inputs["emb_table"], inputs["W0"], inputs["b0"], inputs["W1"],
        inputs["b1"], inputs["Wh"], inputs["bh"])

    if "prog" not in _PROGRAM_CACHE:
        _PROGRAM_CACHE["prog"] = _build_program()
    nc = _PROGRAM_CACHE["prog"]

    from concourse.bass_utils import run_bass_kernel_spmd
    res = run_bass_kernel_spmd(nc, per_core, core_ids=list(range(N_CORES)))

    logp = np.zeros((N_BATCHES, N_ACTIONS), np.float32)
    for core in range(N_CORES):
        out = np.asarray(res.results[core]["out"], np.float32)
        for s in range(NSUB):
            sh = core * NSUB + s
            v = out_map[sh] >= 0
            logp[out_map[sh][v]] = out[s][v]
    return (logp,)



# revision 24
# speedup vs baseline: 1.8881x; 1.2465x over previous
"""Trainium2 Bass kernel for nn_Commnet (gnn_message_passing).

kernel(**inputs) takes FULL unsharded numpy inputs, returns (logp [4096,16],)
computed across 8 NeuronCores (SPMD single program; per-core structure is
carried entirely in input tensors).

Sharding: batches bin-packed into 32 sub-shards (4/core); each sub-shard =
10 batch-groups of 16 batch slots padded to exactly 512 agent slots, so every
512-agent matmul block has a static 16-batch selector window -> the program
is identical on all cores.

EmbeddingBag(mean): table cast to bf16 pre-scaled by 1/8 on host (exact);
SWDGE transpose-mode dma_gather (bag-major int16 idx stream wrapped in 16
partitions, replicated across the 8 Q7 cores) fetches word rows directly
dim-major ([128, 2, nwords]), round-robined over 4 SWDGE queues so the
per-512B-descriptor ring drain overlaps; the bag-sum is a DVE group-8
tensor_reduce straight into the emb dim-tiles.

Middle layers: emb' = relu(W_l@emb - W_r@(emb*recip) + R'@sel) where
R' = W_r@m + b (x) (len-0.99999) folds expansion+bias; sel is a banded
one-hot*recip selector (1 extra k-tile per block). m = segment sum via DVE
group-sum(8) -> PE transpose -> banded matmuls into disjoint PSUM windows.
Padded agents stay exactly 0 through all layers.
"""

from contextlib import ExitStack

import numpy as np
import ml_dtypes

import concourse.bass as bass
import concourse.bacc as bacc
import concourse.tile as tile
from concourse import mybir
from concourse.masks import make_identity

N_WORDS = 32000
EDIM = 256
N_AGENTS = 131072
BAG = 8
N_BATCHES = 4096
N_ACTIONS = 16
NLAYERS = 3

N_CORES = 8
NSUB = 4                  # sub-shards per core
NSHARD = N_CORES * NSUB   # 32
NBG = 9                   # batch-groups per sub-shard
BG_BATCHES = 16
BG_AGENTS = 512
B_S = NBG * BG_BATCHES    # 144
A_S = NBG * BG_AGENTS     # 4608
G = 4                     # segsum group size (batches padded to mult of G)
NG = A_S // G             # 1152
NCHUNK = NG // 128        # 9 (chunk c == batch-group c)
W2 = 2 * BG_BATCHES       # 32 (psum row-window width, chunk pairs share)
P = 128
SECOND = B_S - P          # 16 batches in the second psum column half
ZPAD_ROW = N_WORDS
DT = mybir.dt
AF = mybir.ActivationFunctionType
OP = mybir.AluOpType

_PROGRAM_CACHE = {}
_DEBUG = False


# ================================================================ host prep
def _pack_batches(counts):
    padded = ((counts + G - 1) // G) * G
    ngroups = NSHARD * NBG
    cap = np.full(ngroups, BG_AGENTS, dtype=np.int64)
    slots = np.full(ngroups, BG_BATCHES, dtype=np.int64)
    members = [[] for _ in range(ngroups)]
    for b in np.argsort(-padded, kind="stable"):
        ok = (cap >= padded[b]) & (slots > 0)
        if not ok.any():
            raise RuntimeError("bin packing failed")
        g = int(np.argmax(np.where(ok, cap, -1)))
        members[g].append(int(b))
        cap[g] -= padded[b]
        slots[g] -= 1
    return members, padded


def _build_host_inputs(x, batch_idx, batch_len, emb_table, W0, b0, W1, b1,
                       Wh, bh):
    bf16 = ml_dtypes.bfloat16
    x = np.asarray(x, dtype=np.int64)
    batch_idx = np.asarray(batch_idx, dtype=np.int64)
    batch_len64 = np.asarray(batch_len, dtype=np.float64)

    counts = np.bincount(batch_idx, minlength=N_BATCHES).astype(np.int64)
    starts = np.concatenate([[0], np.cumsum(counts)[:-1]])
    members, padded = _pack_batches(counts)

    # transpose-mode dma_gather int16 indices, bag-major stream: stream
    # position i = slot*8 + k; position i lives at [i%16 (+16c), i//16].
    idx_all = np.zeros((NSHARD, P, BAG * A_S // 16), dtype=np.int16)
    xsel_all = np.zeros((NSHARD, P, A_S), dtype=bf16)
    recip_all = np.zeros((NSHARD, P, A_S), dtype=bf16)
    s2_all = np.zeros((NSHARD, P, NCHUNK, W2), dtype=bf16)
    lenm1_all = np.zeros((NSHARD, 1, B_S), dtype=bf16)
    gcnt_all = np.full((NSHARD, 1, NBG // 2), BG_AGENTS * 2, dtype=np.int32)
    out_map = np.full((NSHARD, B_S), -1, dtype=np.int64)
    dbg_slots = []
    recip_f = (1.0 / (batch_len64 - 0.99999)).astype(np.float32)

    for sh in range(NSHARD):
        idx_flat = np.full((BAG, A_S), ZPAD_ROW, dtype=np.int64)
        a_of_slot = np.full(A_S, -1, dtype=np.int64)
        b_of_slot = np.full(A_S, -1, dtype=np.int64)
        grp_content = np.zeros(NBG, np.int64)
        for bg in range(NBG):
            pos = bg * BG_AGENTS
            for sl, b in enumerate(members[sh * NBG + bg]):
                lb = bg * BG_BATCHES + sl
                out_map[sh, lb] = b
                lenm1_all[sh, 0, lb] = np.float32(batch_len64[b] - 0.99999)
                n = int(counts[b])
                a_of_slot[pos:pos + n] = np.arange(starts[b], starts[b] + n)
                b_of_slot[pos:pos + n] = lb
                pos += int(padded[b])
            grp_content[bg] = pos - bg * BG_AGENTS
        slots = np.nonzero(a_of_slot >= 0)[0]
        ags = a_of_slot[slots]
        for k in range(BAG):
            idx_flat[k, slots] = x[ags * BAG + k]

        # bag-major stream: i = slot*8 + k -> wrap [16, i//16], repl x8
        stream = idx_flat.T.reshape(-1)            # [A_S * BAG]
        blk = stream.reshape(-1, 16).T             # [16, A_S * BAG / 16]
        for c in range(8):
            idx_all[sh, 16 * c:16 * (c + 1), :] = blk

        lb_real = b_of_slot[slots]
        rec = recip_f[out_map[sh, lb_real]]
        recip_row = np.zeros(A_S, np.float32)
        recip_row[slots] = rec
        recip_all[sh] = np.broadcast_to(recip_row.astype(bf16), (P, A_S))

        j_of_slot = slots // BG_AGENTS
        w0_al = np.where(j_of_slot >= 8, 128, (j_of_slot // 4) * 64)
        r = lb_real - w0_al
        assert (r >= 0).all() and (r < P).all()
        xs = np.zeros((P, A_S), np.float32)
        xs[r, slots] = rec
        xsel_all[sh] = xs.astype(bf16)

        dbg_slots.append((a_of_slot.copy(), b_of_slot.copy()))
        g_b = b_of_slot[::G]
        for c in range(NCHUNK):
            gl = np.arange(P)
            gb = g_b[c * P + gl]
            v = gb >= 0
            r0 = 32 * (c // 2) if c < 8 else P
            w = gb[v] - r0
            assert (w >= 0).all() and (w < W2).all()
            s2_all[sh][gl[v], c, w] = 1.0

    W0 = np.asarray(W0, np.float32)
    W1 = np.asarray(W1, np.float32)
    wl = np.stack([W0[:, :EDIM].T, W1[:, :EDIM].T])   # [layer, 256k, 256d]
    wr = np.stack([W0[:, EDIM:].T, W1[:, EDIM:].T])

    def tiles(w):  # [2,256,256] -> [128, 2(layer), 2(kt), 2(dt), 128]
        t = w.reshape(2, 2, P, 2, P).transpose(2, 0, 1, 3, 4)
        return np.ascontiguousarray(t).astype(bf16)

    host = {
        "table": np.concatenate(
            [np.asarray(emb_table, np.float32) / 8.0,
             np.zeros((1, EDIM), np.float32)], 0).astype(bf16),
        "wlT": tiles(wl),
        "wrTn": tiles(-wr),
        "wrT": tiles(wr),
        "bias": np.ascontiguousarray(
            np.stack([np.asarray(b0, np.float32), np.asarray(b1, np.float32)])
            .reshape(2, 2, P).transpose(1, 0, 2)[None]  # wrong axis order?
        ),
        "whT": np.ascontiguousarray(
            np.asarray(Wh, np.float32).T.reshape(2, P, N_ACTIONS)
            .transpose(1, 0, 2)).astype(bf16),
        "bh": np.asarray(bh, np.float32).reshape(1, N_ACTIONS).astype(bf16),
        "ones_b": np.ones((1, B_S), bf16),
    }
    # bias layout: [1, 2(layer), 2(dt), 128]
    bias = np.stack([np.asarray(b0, np.float32),
                     np.asarray(b1, np.float32)]).reshape(2, 2, P)
    host["bias"] = bias[None].astype(bf16)

    per_core = []
    for core in range(N_CORES):
        s0 = core * NSUB
        m = dict(host)
        m["idx"] = idx_all[s0:s0 + NSUB]
        m["xsel"] = xsel_all[s0:s0 + NSUB]
        m["recipb"] = recip_all[s0:s0 + NSUB]
        m["s2"] = s2_all[s0:s0 + NSUB]
        m["lenm1"] = lenm1_all[s0:s0 + NSUB]
        per_core.append(m)
    return per_core, out_map, dbg_slots


# ============================================================ device program
def _build_program():
    nc = bacc.Bacc("TRN2", num_swdge_queues=4)
    bf, f32, i32 = DT.bfloat16, DT.float32, DT.int32

    i16 = DT.int16
    table = nc.dram_tensor("table", [N_WORDS + 1, EDIM], bf,
                           kind="ExternalInput")
    idx_d = nc.dram_tensor("idx", [NSUB, P, BAG * A_S // 16], i16,
                           kind="ExternalInput")
    xsel_d = nc.dram_tensor("xsel", [NSUB, P, A_S], bf, kind="ExternalInput")
    recip_d = nc.dram_tensor("recipb", [NSUB, P, A_S], bf,
                             kind="ExternalInput")
    s2_d = nc.dram_tensor("s2", [NSUB, P, NCHUNK, W2], bf,
                          kind="ExternalInput")
    lenm1_d = nc.dram_tensor("lenm1", [NSUB, 1, B_S], bf,
                             kind="ExternalInput")
    wlT_d = nc.dram_tensor("wlT", [P, 2, 2, 2, P], bf, kind="ExternalInput")
    wrTn_d = nc.dram_tensor("wrTn", [P, 2, 2, 2, P], bf, kind="ExternalInput")
    wrT_d = nc.dram_tensor("wrT", [P, 2, 2, 2, P], bf, kind="ExternalInput")
    bias_d = nc.dram_tensor("bias", [1, 2, 2, P], bf, kind="ExternalInput")
    whT_d = nc.dram_tensor("whT", [P, 2, N_ACTIONS], bf,
                           kind="ExternalInput")
    bh_d = nc.dram_tensor("bh", [1, N_ACTIONS], bf, kind="ExternalInput")
    ones_d = nc.dram_tensor("ones_b", [1, B_S], bf, kind="ExternalInput")
    out_d = nc.dram_tensor("out", [NSUB, B_S, N_ACTIONS], f32,
                           kind="ExternalOutput")
    if _DEBUG:
        dbg_emb0 = nc.dram_tensor("dbg_emb0", [2, P, A_S], bf,
                                  kind="ExternalOutput")
        dbg_emb1 = nc.dram_tensor("dbg_emb1", [2, P, A_S], bf,
                                  kind="ExternalOutput")
        dbg_mT = nc.dram_tensor("dbg_mT", [P, 512], f32,
                                kind="ExternalOutput")
        dbg_x2 = nc.dram_tensor("dbg_x2", [2, P, A_S], bf,
                                kind="ExternalOutput")
        dbg_r = nc.dram_tensor("dbg_r", [P, 2 * B_S], f32,
                               kind="ExternalOutput")
        dbg_h = nc.dram_tensor("dbg_h", [N_ACTIONS, B_S], f32,
                               kind="ExternalOutput")

    with tile.TileContext(nc) as tc, ExitStack() as ctx:
        consts = ctx.enter_context(tc.tile_pool(name="consts", bufs=1))
        wpool = ctx.enter_context(tc.tile_pool(name="wpool", bufs=1))
        gpool = ctx.enter_context(tc.tile_pool(name="gath", bufs=2))
        e0pool = ctx.enter_context(tc.tile_pool(name="emb0", bufs=2))
        e1pool = ctx.enter_context(tc.tile_pool(name="emb1", bufs=2))
        e2pool = ctx.enter_context(tc.tile_pool(name="emb2", bufs=1))
        xpool = ctx.enter_context(tc.tile_pool(name="x2p", bufs=1))
        spool = ctx.enter_context(tc.tile_pool(name="small", bufs=2))
        main_ps = ctx.enter_context(
            tc.tile_pool(name="mps", bufs=4, space="PSUM"))
        tp_ps = ctx.enter_context(
            tc.tile_pool(name="tps", bufs=2, space="PSUM"))
        sm_ps = ctx.enter_context(
            tc.tile_pool(name="sps", bufs=2, space="PSUM"))

        ident = consts.tile([P, P], f32, tag="ident", name="ident")
        make_identity(nc, ident[:])

        wlT = wpool.tile([P, 2, 2, 2, P], bf, tag="wlT", name="wlT")
        nc.sync.dma_start(wlT[:], wlT_d[:])
        wrTn = wpool.tile([P, 2, 2, 2, P], bf, tag="wrTn", name="wrTn")
        nc.sync.dma_start(wrTn[:], wrTn_d[:])
        wrT = wpool.tile([P, 2, 2, 2, P], bf, tag="wrT", name="wrT")
        nc.sync.dma_start(wrT[:], wrT_d[:])
        bias_sb = wpool.tile([1, 2, 2, P], bf, tag="bias", name="bias")
        nc.sync.dma_start(bias_sb[:], bias_d[:])
        whT = wpool.tile([P, 2, N_ACTIONS], bf, tag="whT", name="whT")
        nc.sync.dma_start(whT[:], whT_d[:])
        bh_sb = wpool.tile([1, N_ACTIONS], bf, tag="bh", name="bh")
        nc.sync.dma_start(bh_sb[:], bh_d[:])
        ones_sb = wpool.tile([1, B_S], bf, tag="ones", name="ones")
        nc.sync.dma_start(ones_sb[:], ones_d[:])

        def tpack(src_aps, dst_ap):
            """PE-transpose [p<=128, w<=128] fp32 APs into one psum bank,
            then one ACT copy (w/ cast) into dst_ap (columns concatenated).
            Each src must have 128 partitions."""
            ps = tp_ps.tile([P, 512], f32, tag="tpack", name="tpack")
            col = 0
            for a in src_aps:
                w = a.shape[-1]
                nc.tensor.transpose(ps[:, col:col + w], a, ident[:])
                col += w
            nc.scalar.activation(dst_ap, ps[:, :col], AF.Copy)

        NOPW = 1024               # words per transpose-gather op
                                  # (4096 wedges the device in transpose mode)
        NGOP = A_S * BAG // NOPW  # 40 gather ops per sub-shard
        for s in range(NSUB):
            # -------- Phase A: transpose-gather + DVE group-8 bag-sum
            idx_sb = gpool.tile([P, BAG * A_S // 16], i16, tag="idx",
                                name="idx")
            nc.sync.dma_start(idx_sb[:], idx_d[s])
            xsel = gpool.tile([P, A_S], bf, tag="xsel", name="xsel", bufs=1)
            nc.sync.dma_start(xsel[:], xsel_d[s])
            recipb = gpool.tile([P, A_S], bf, tag="recipb", name="recipb", bufs=1)
            nc.sync.dma_start(recipb[:], recip_d[s])
            s2_sb = gpool.tile([P, NCHUNK, W2], bf, tag="s2", name="s2")
            nc.sync.dma_start(s2_sb[:], s2_d[s])
            lenm1 = gpool.tile([1, B_S], bf, tag="lenm1", name="lenm1")
            nc.sync.dma_start(lenm1[:], lenm1_d[s])

            emb = [None] * NLAYERS
            emb[0] = e0pool.tile([P, 2, A_S], bf, tag="emb0", name="emb0")
            emb[1] = e1pool.tile([P, 2, A_S], bf, tag="emb1", name="emb1")
            emb[2] = e2pool.tile([P, 2, A_S], bf, tag="emb2", name="emb2")

            for q in range(NGOP):
                slab = gpool.tile([P, 2, NOPW], bf, tag="slab",
                                  name="slab", bufs=6)
                nc.gpsimd.dma_gather(
                    out_ap=slab[:], in_ap=table[:],
                    idxs_ap=idx_sb[:, q * (NOPW // 16):
                                   (q + 1) * (NOPW // 16)],
                    num_idxs=NOPW, num_idxs_reg=NOPW, elem_size=EDIM,
                    transpose=True, single_packet=False,
                    queue_num=q % 4)
                na = NOPW // BAG  # 128 agent slots per op
                with nc.allow_low_precision(
                        reason="bag-sum of 8 bf16 rows; ~0.3% rms, "
                               "well under the 2e-2 budget"):
                    nc.vector.tensor_reduce(
                        emb[0][:, :, q * na:(q + 1) * na],
                        slab[:].rearrange("p t (a g) -> p t a g", g=BAG),
                        axis=mybir.AxisListType.X, op=OP.add)

            if _DEBUG and s == 0:
                for t in range(2):
                    nc.sync.dma_start(dbg_emb0[t], emb[0][:, t, :])

            # ---------------- helpers
            def segsum(src):
                """src = bf16 [P, 2, A_S] -> m^T psum [P, 512] f32:
                cols 0:256 = batches 0:128, cols 256:512 = batches 128:160
                (on partitions 0:32)."""
                grp = spool.tile([P, 2, NG], f32, tag="grp", name="grp",
                                 bufs=1)
                nc.vector.tensor_reduce(
                    grp[:],
                    src[:].rearrange("p t (g e) -> p t g e", e=G),
                    axis=mybir.AxisListType.X, op=OP.add)
                gt = spool.tile([P, NCHUNK * EDIM], bf, tag="gt", name="gt")
                for c in range(NCHUNK):
                    tpack([grp[:, t, c * P:(c + 1) * P] for t in range(2)],
                          gt[:, c * EDIM:(c + 1) * EDIM])
                m_ps = sm_ps.tile([P, 512], f32, tag="sps", name="sps")
                for c in range(NCHUNK):
                    r0 = 32 * (c // 2) if c < 8 else 0
                    dst = (m_ps[r0:r0 + W2, 0:EDIM] if c < 8
                           else m_ps[0:W2, EDIM:2 * EDIM])
                    nc.tensor.matmul(dst, lhsT=s2_sb[:, c, :],
                                     rhs=gt[:, c * EDIM:(c + 1) * EDIM],
                                     start=(c % 2 == 0 or c == 8),
                                     stop=(c % 2 == 1 or c == 8),
                                     skip_group_check=True,
                                     tile_position=(0, r0))
                return m_ps

            def m_to_sbuf(m_ps):
                mT = spool.tile([P, 512], f32, tag="mT", name="mT")
                nc.scalar.activation(mT[:, 0:EDIM], m_ps[:, 0:EDIM], AF.Copy)
                nc.scalar.activation(mT[0:SECOND, EDIM:2 * EDIM],
                                     m_ps[0:SECOND, EDIM:2 * EDIM], AF.Copy)
                return mT

            def m_dimmajor(mT_sb):
                """m^T sbuf -> mdm bf16 [P, 2(dt), B_S] (dim-major m)."""
                ps = sm_ps.tile([P, 512], f32, tag="sps", name="sps")
                for t in range(2):
                    nc.tensor.transpose(ps[:, t * B_S:t * B_S + P],
                                        mT_sb[:, t * P:(t + 1) * P],
                                        ident[:])
                    nc.tensor.transpose(
                        ps[:, t * B_S + P:t * B_S + B_S],
                        mT_sb[0:SECOND, EDIM + t * P:EDIM + (t + 1) * P],
                        ident[0:SECOND, 0:SECOND])
                out = spool.tile([P, 2 * B_S], bf, tag="mdm", name="mdm")
                nc.scalar.activation(out[:], ps[:, 0:2 * B_S], AF.Copy)
                return out

            # ---------------- layers 0, 1
            for i in range(2):
                x2 = xpool.tile([P, 2, A_S], bf, tag="x2", name="x2")
                for t in range(2):
                    nc.vector.tensor_tensor(out=x2[:, t, :],
                                            in0=emb[i][:, t, :],
                                            in1=recipb[:], op=OP.mult)
                m_ps = segsum(emb[i])
                mT_sb_dbg = m_to_sbuf(m_ps)
                if _DEBUG and s == 0 and i == 0:
                    for t in range(2):
                        nc.sync.dma_start(dbg_x2[t], x2[:, t, :])
                    nc.sync.dma_start(dbg_mT[:], mT_sb_dbg[:])
                mdm = m_dimmajor(mT_sb_dbg)
                r_ps = sm_ps.tile([P, 512], f32, tag="sps", name="sps")
                for dt in range(2):
                    sl = r_ps[:, dt * B_S:(dt + 1) * B_S]
                    for kt in range(2):
                        nc.tensor.matmul(
                            sl, lhsT=wrT[:, i, kt, dt, :],
                            rhs=mdm[:, kt * B_S:(kt + 1) * B_S],
                            start=(kt == 0), stop=False)
                    nc.tensor.matmul(sl, lhsT=bias_sb[:, i, dt, :],
                                     rhs=lenm1[:], start=False, stop=True)
                r_sb = spool.tile([P, 2 * B_S], f32, tag="r_sb", name="r_sb")
                nc.scalar.activation(r_sb[:], r_ps[:, 0:2 * B_S], AF.Copy)
                if _DEBUG and s == 0 and i == 0:
                    nc.sync.dma_start(dbg_r[:], r_sb[:])
                # R^T at alignments 0 / 64 / 128 -> rt [P, 3, 256] bf16
                rt = spool.tile([P, 3, EDIM], bf, tag="rt", name="rt")
                nc.vector.memset(rt[:], 0.0)
                rt_ps = sm_ps.tile([P, 512], f32, tag="sps", name="sps")
                for dt in range(2):
                    nc.tensor.transpose(
                        rt_ps[:, dt * P:(dt + 1) * P],
                        r_sb[:, dt * B_S:dt * B_S + P], ident[:])
                nc.scalar.activation(rt[:, 0, :], rt_ps[:, 0:EDIM], AF.Copy)
                rt_ps2 = sm_ps.tile([P, 512], f32, tag="sps", name="sps")
                for dt in range(2):
                    nc.tensor.transpose(
                        rt_ps2[0:B_S - 64, dt * P:(dt + 1) * P],
                        r_sb[:, dt * B_S + 64:dt * B_S + B_S], ident[:])
                    nc.tensor.transpose(
                        rt_ps2[0:SECOND, EDIM + dt * P:EDIM + dt * P + P],
                        r_sb[:, dt * B_S + P:dt * B_S + B_S], ident[:])
                nc.scalar.activation(rt[0:B_S - 64, 1, :],
                                     rt_ps2[0:B_S - 64, 0:EDIM], AF.Copy)
                nc.scalar.activation(rt[0:SECOND, 2, :],
                                     rt_ps2[0:SECOND, EDIM:2 * EDIM], AF.Copy)
                # main matmuls
                for j in range(NBG):
                    al = min(j // 4, 2)
                    js = slice(j * BG_AGENTS, (j + 1) * BG_AGENTS)
                    for dt in range(2):
                        ps = main_ps.tile([P, BG_AGENTS], f32, tag="main", name="main")
                        for kt in range(2):
                            nc.tensor.matmul(
                                ps[:], lhsT=wlT[:, i, kt, dt, :],
                                rhs=emb[i][:, kt, js],
                                start=(kt == 0), stop=False)
                        for kt in range(2):
                            nc.tensor.matmul(
                                ps[:], lhsT=wrTn[:, i, kt, dt, :],
                                rhs=x2[:, kt, js], start=False, stop=False)
                        nc.tensor.matmul(
                            ps[:], lhsT=rt[:, al, dt * P:(dt + 1) * P],
                            rhs=xsel[:, js], start=False, stop=True)
                        nc.scalar.activation(emb[i + 1][:, dt, js], ps[:],
                                             AF.Relu)
                if _DEBUG and s == 0 and i == 0:
                    for t in range(2):
                        nc.sync.dma_start(dbg_emb1[t], emb[1][:, t, :])

            # ---------------- final segsum + head + log_softmax
            m_ps = segsum(emb[2])
            mdm = m_dimmajor(m_to_sbuf(m_ps))
            h_ps = sm_ps.tile([P, 512], f32, tag="sps", name="sps")
            hsl = h_ps[0:N_ACTIONS, 0:B_S]
            for kt in range(2):
                nc.tensor.matmul(hsl, lhsT=whT[:, kt, :],
                                 rhs=mdm[:, kt * B_S:(kt + 1) * B_S],
                                 start=(kt == 0), stop=False)
            nc.tensor.matmul(hsl, lhsT=bh_sb[:], rhs=ones_sb[:],
                             start=False, stop=True)
            h_sb = spool.tile([N_ACTIONS, B_S], f32, tag="h_sb", name="h_sb")
            nc.scalar.activation(h_sb[:], hsl, AF.Copy)
            if _DEBUG and s == 0:
                nc.sync.dma_start(dbg_h[:], h_sb[:])
            lg_ps = sm_ps.tile([P, 512], f32, tag="sps", name="sps")
            nc.tensor.transpose(lg_ps[:, 0:N_ACTIONS], h_sb[:, 0:P],
                                ident[0:N_ACTIONS, 0:N_ACTIONS])
            nc.tensor.transpose(lg_ps[0:SECOND, N_ACTIONS:2 * N_ACTIONS],
                                h_sb[:, P:B_S],
                                ident[0:N_ACTIONS, 0:N_ACTIONS])
            lg = spool.tile([P, 2 * N_ACTIONS], f32, tag="lg_sb", name="lg_sb")
            nc.scalar.activation(lg[:, 0:N_ACTIONS], lg_ps[:, 0:N_ACTIONS],
                                 AF.Copy)
            nc.scalar.activation(lg[0:SECOND, N_ACTIONS:2 * N_ACTIONS],
                                 lg_ps[0:SECOND, N_ACTIONS:2 * N_ACTIONS],
                                 AF.Copy)
            for part in range(2):
                rows = P if part == 0 else B_S - P
                src = lg[0:rows, part * N_ACTIONS:(part + 1) * N_ACTIONS]
                mx = spool.tile([P, 1], f32, tag="mx", name="mx")
                nc.vector.tensor_reduce(mx[0:rows, :], src,
                                        axis=mybir.AxisListType.X,
                                        op=OP.max)
                shv = spool.tile([P, N_ACTIONS], f32, tag="shift", name="shift")
                nc.vector.tensor_tensor(
                    out=shv[0:rows, :], in0=src,
                    in1=mx[0:rows, :].to_broadcast([rows, N_ACTIONS]),
                    op=OP.subtract)
                ex = spool.tile([P, N_ACTIONS], f32, tag="ex", name="ex")
                se = spool.tile([P, 1], f32, tag="se", name="se")
                nc.scalar.activation(ex[0:rows, :], shv[0:rows, :], AF.Exp,
                                     accum_out=se[0:rows, :])
                lse = spool.tile([P, 1], f32, tag="lse", name="lse")
                nc.scalar.activation(lse[0:rows, :], se[0:rows, :], AF.Ln)
                res = spool.tile([P, N_ACTIONS], f32, tag="res", name="res")
                nc.vector.tensor_tensor(
                    out=res[0:rows, :], in0=shv[0:rows, :],
                    in1=lse[0:rows, :].to_broadcast([rows, N_ACTIONS]),
                    op=OP.subtract)
                nc.sync.dma_start(out_d[s, part * P:part * P + rows, :],
                                  res[0:rows, :])
    nc.compile()
    return nc


# ================================================================== kernel
def kernel(**inputs):
    per_core, out_map, _ = _build_host_inputs(
        inputs["x"], inputs["batch_idx"], inputs["batch_len"],
        inputs["emb_table"], inputs["W0"], inputs["b0"], inputs["W1"],
        inputs["b1"], inputs["Wh"], inputs["bh"])

    if "prog" not in _PROGRAM_CACHE:
        _PROGRAM_CACHE["prog"] = _build_program()
    nc = _PROGRAM_CACHE["prog"]

    from concourse.bass_utils import run_bass_kernel_spmd
    res = run_bass_kernel_spmd(nc, per_core, core_ids=list(range(N_CORES)))

    logp = np.zeros((N_BATCHES, N_ACTIONS), np.float32)
    for core in range(N_CORES):
        out = np.asarray(res.results[core]["out"], np.float32)
        for s in range(NSUB):
            sh = core * NSUB + s
            v = out_map[sh] >= 0
            logp[out_map[sh][v]] = out[s][v]
    return (logp,)

